# revision 13
# baseline (speedup 1.0000x reference)
"""Causal self-attention (B=2, S=2048, H=1024, 16 heads) on 8 trn2 NeuronCores.

Sharding: core c handles batch b = c // 4 and head-group g = c % 4
(4 heads x d=64 = 256 output columns). Fully parallel, no collectives.

v3 pipeline (per core, all matmuls f32r = TF32-class):
  - x -> xT via f32r PE transposes (4 per PSUM bank, one [128,512] DVE evac)
  - QT/KT = W^T xT + b (d on partitions), V natural with [1 | V_h] interleave
  - scores^T per head PAIR packed into the PE array via tile_position
    (d=64 contraction fills half the rows); one exp over [128,1024] PSUM
  - causal diag masking: one half on DVE (0/1 mask mul), one on GPSIMD
  - PV: [1|V_h]^T @ expT accumulated in PSUM -> [65, 512]; row 0 = softmax
    denominator; tail = reciprocal + SBUF DMA-broadcast + one DVE multiply;
    output stored TRANSPOSED [256, 2048] per core, host transposes back
  - emission: software-pipelined score stream (2 ahead of PV) with the
    remaining projection/V/transpose work woven in as fine-grained fillers
    so ACT's exp pipeline saturates from ~10us onward
"""

from collections import deque

import numpy as np

import concourse.bacc as bacc
import concourse.mybir as mybir
from concourse.tile import TileContext
from concourse.bass_utils import run_bass_kernel_spmd
from concourse.masks import make_identity

B, S, H, NH, D = 2, 2048, 1024, 16, 64
P = 128
NCORES = 8
NHL = NH // 4            # 4 heads per core
HGD = NHL * D            # 256 output cols per core
HC = H // P              # 8 contraction chunks
SC = S // P              # 16 sequence chunks of 128
QC = S // 512            # 4 query chunks of 512
KC = S // P              # 16 key chunks of 128
DC = HGD // P            # 2 partition chunks of QT/KT

fp32 = mybir.dt.float32
f32r = mybir.dt.float32r
bf16 = mybir.dt.bfloat16
fp8 = mybir.dt.float8e4
AF = mybir.ActivationFunctionType
ALU = mybir.AluOpType
DR = mybir.MatmulPerfMode.DoubleRow

_CACHE = {}
LAST_RESULTS = None


def _emit(nc):
    x = nc.declare_dram_parameter("x", [S, H], f32r, isOutput=False)
    wq = nc.declare_dram_parameter("wq", [H, HGD], f32r, isOutput=False)
    wk = nc.declare_dram_parameter("wk", [H, HGD], f32r, isOutput=False)
    wv = nc.declare_dram_parameter("wv", [H, HGD], f32r, isOutput=False)
    bq = nc.declare_dram_parameter("bq", [HGD], fp32, isOutput=False)
    bk = nc.declare_dram_parameter("bk", [HGD], fp32, isOutput=False)
    bv = nc.declare_dram_parameter("bv", [HGD], fp32, isOutput=False)
    mask = nc.declare_dram_parameter("mask", [S], fp32, isOutput=False)
    # transposed output: host does the final [HGD, S] -> [S, HGD] transpose
    out = nc.declare_dram_parameter("out", [HGD, S], fp32, isOutput=True)

    scale = float(1.0 / np.sqrt(np.float32(D)))

    with TileContext(nc) as tc:
        with tc.tile_pool(name="const", bufs=1) as const, \
             tc.tile_pool(name="big", bufs=1) as big:

            # ---- big tiles ----
            xT = big.tile([P, HC, S], f32r, tag="xT")
            QT = big.tile([P, DC, S], f32r, tag="QT")
            KT = big.tile([P, DC, S], f32r, tag="KT")
            VS, VOFF = 128, 64  # per-head [1 | zeros*63 | V] stationary layout
            # fp8 V (all key blocks) for DoubleRow PV on q rows >= 512;
            # bf16 V (key blocks 0..3) for the accuracy-critical first 512 rows
            Vt8 = big.tile([P, SC, NHL * VS], fp8, tag="Vt8")
            Vt8_4 = Vt8.rearrange("p sc (h c) -> p sc h c", c=VS)
            Vt16 = big.tile([P, 4, NHL * VS], bf16, tag="Vt16")
            Vt16_4 = Vt16.rearrange("p sc (h c) -> p sc h c", c=VS)

            with tc.tile_pool(name="xin", bufs=4) as xin, \
                 tc.tile_pool(name="et8", bufs=3) as et8p, \
                 tc.tile_pool(name="et16", bufs=3) as et16p, \
                 tc.tile_pool(name="rcp", bufs=2) as rcpp, \
                 tc.tile_pool(name="cnorm", bufs=2) as cnp, \
                 tc.tile_pool(name="psW", bufs=2, space="PSUM") as psW, \
                 tc.tile_pool(name="psE", bufs=1, space="PSUM") as psE:

                # ---------- early loads: x first (PE depends on it) ----------
                def emit_xload(sc):
                    xt = xin.tile([P, H], f32r, tag="xt", name="xt")
                    nc.sync.dma_start(xt[:], x[sc * P:(sc + 1) * P, :])
                    return xt

                xt0 = xin.tile([P, H], f32r, tag="xt", name="xt")
                nc.sync.dma_start(xt0[:, 0:512], x[0:P, 0:512])
                nc.sync.dma_start(xt0[:, 512:H], x[0:P, 512:H])
                early_xt = [xt0] + [emit_xload(sc) for sc in range(1, 4)]

                ident_f = const.tile([P, P], fp32, tag="identf")
                make_identity(nc, ident_f)
                ident_r = const.tile([P, P], f32r, tag="identr")
                nc.vector.tensor_copy(ident_r[:], ident_f[:])
                # 0/1 causal masks: cm[:, j, f] = (f - p >= -j*128)
                cm = const.tile([P, 4, 512], mybir.dt.bfloat16, tag="cmask")
                nc.gpsimd.memset(cm[:], 1.0)
                for j in range(4):
                    nc.gpsimd.affine_select(
                        out=cm[:, j, :], in_=cm[:, j, :],
                        compare_op=ALU.is_ge, fill=0.0,
                        base=-j * P, pattern=[[1, 512]], channel_multiplier=-1)
                # only the 128-wide partial band [j*128, j*128+128) is ever
                # applied; keep an fp8 copy of those bands for the fp8 path
                cm8 = const.tile([P, 4, P], fp8, tag="cmask8")
                for j in range(4):
                    nc.gpsimd.tensor_copy(cm8[:, j, :], cm[:, j, j * P:(j + 1) * P])

                # weights / biases / mask (needed ~15us in)
                wq_t = big.tile([P, HC, HGD], f32r, tag="wq")
                wk_t = big.tile([P, HC, HGD], f32r, tag="wk")
                wv_t = big.tile([P, HC, HGD], f32r, tag="wv")
                nc.sync.dma_start(wq_t[:], wq[:].rearrange("(hc p) n -> p hc n", p=P))
                nc.sync.dma_start(wk_t[:], wk[:].rearrange("(hc p) n -> p hc n", p=P))
                nc.sync.dma_start(wv_t[:], wv[:].rearrange("(hc p) n -> p hc n", p=P))
                bq_t = const.tile([P, DC], fp32, tag="bq")
                bk_t = const.tile([P, DC], fp32, tag="bk")
                nc.sync.dma_start(bq_t[:], bq[:].rearrange("(dc p) -> p dc", p=P))
                nc.sync.dma_start(bk_t[:], bk[:].rearrange("(dc p) -> p dc", p=P))
                bv_b = const.tile([P, HGD], fp32, tag="bv")
                nc.gpsimd.dma_start(bv_b[:], bv[None, :].to_broadcast([P, HGD]))
                bv4 = bv_b.rearrange("p (h c) -> p h c", c=D)
                mask_t = const.tile([P, KC], fp32, tag="mask")
                nc.sync.dma_start(mask_t[:], mask[:].rearrange("(kc p) -> p kc", p=P))
                # fp8-path exp bias: mask - 3. Softmax is shift-invariant (the
                # ones-column denominator absorbs e^-3), and the shift moves
                # fp8e4 overflow (448) from ~6 sigma scores out to ~9 sigma.
                mask2_t = const.tile([P, KC], fp32, tag="mask2")
                neg3_c = const.tile([P, 1], fp32, tag="neg3")
                nc.vector.memset(neg3_c[:], -3.0)
                nc.vector.tensor_scalar_add(mask2_t[:], mask_t[:],
                                            neg3_c[:, 0:1])

                zero_c = const.tile([P, 1], fp32, tag="zero")
                nc.vector.memset(zero_c[:], 0.0)
                ones_c = const.tile([P, 1], fp32, tag="ones")
                nc.vector.memset(ones_c[:], 1.0)

                def emit_xtr(xt, sc, hg):  # 4 transposes + 1 evac
                    tp = psW.tile([P, 512], f32r, tag="pp", name="tp")
                    for j in range(4):
                        hc = hg * 4 + j
                        nc.tensor.matmul(
                            tp[:, j * P:(j + 1) * P],
                            xt[:, hc * P:(hc + 1) * P], ident_r[:],
                            is_transpose=True, start=(j == 0), stop=(j == 3),
                            skip_group_check=True)
                    nc.vector.tensor_copy(
                        xT[:, hg * 4:(hg + 1) * 4, sc * P:(sc + 1) * P],
                        tp.rearrange("p (j c) -> p j c", c=P))

                def proj_closures(W, bias_t, OUT, dc, sq):
                    pp = [None]

                    def mk(hc):
                        def go():
                            if hc == 0:
                                pp[0] = psW.tile([P, 512], fp32, tag="pp", name="pp")
                            nc.tensor.matmul(
                                pp[0][:], W[:, hc, dc * P:(dc + 1) * P],
                                xT[:, hc, sq * 512:(sq + 1) * 512],
                                start=(hc == 0), stop=(hc == HC - 1))
                            if hc == HC - 1:
                                nc.vector.tensor_scalar_add(
                                    OUT[:, dc, sq * 512:(sq + 1) * 512],
                                    pp[0][:], bias_t[:, dc:dc + 1])
                        return go
                    return [mk(hc) for hc in range(HC)]

                def v_closures(scp):  # V for sc pair -> one [128,512] bank
                    pp = [None]

                    def mk(half, hc):
                        def go():
                            if half == 0 and hc == 0:
                                pp[0] = psW.tile([P, 512], fp32, tag="pp", name="pp")
                            sc = scp * 2 + half
                            nc.tensor.matmul(
                                pp[0][:, half * HGD:(half + 1) * HGD],
                                xT[:, hc, sc * P:(sc + 1) * P], wv_t[:, hc, :],
                                start=(half == 0 and hc == 0),
                                stop=(hc == HC - 1), skip_group_check=True)
                            if half == 1 and hc == HC - 1:
                                pp2 = pp[0].rearrange("p (s h c) -> p s h c",
                                                      s=2, c=D)
                                bvb = bv4[:, None, :, :].to_broadcast(
                                    [P, 2, NHL, D])
                                nc.vector.tensor_tensor(
                                    Vt8_4[:, scp * 2:scp * 2 + 2, :,
                                          VOFF:VOFF + D],
                                    pp2, bvb, ALU.add)
                                if scp < 2:
                                    nc.vector.tensor_tensor(
                                        Vt16_4[:, scp * 2:scp * 2 + 2, :,
                                               VOFF:VOFF + D],
                                        pp2, bvb, ALU.add)
                        return go
                    return [mk(h, hc) for h in range(2) for hc in range(HC)]

                # ---------- filler queue with dependency markers ----------
                fillers = deque()
                markers = {}
                done = [0]

                def pull(n):
                    for _ in range(n):
                        if not fillers:
                            return
                        fillers.popleft()()
                        done[0] += 1

                def drain_to(marker):
                    tgt = markers.get(marker, 0)
                    while done[0] < tgt:
                        fillers.popleft()()
                        done[0] += 1

                def add_fillers(closures):
                    fillers.extend(closures)

                def set_marker(name):
                    markers[name] = done[0] + len(fillers)

                # ---------- attention ----------
                # et tiles hold an exp'd score PAIR of key blocks:
                # [P, kc%2, head-of-pair, 512q]. fp8 for q rows >= 512
                # (consumed by DoubleRow PV over the kc pair), bf16 for the
                # accuracy-critical first 512 rows (plain per-kc PV).
                def sc_exp(pr, qc, kc, et):
                    # diagonal tiles (j >= 0): columns f < j*128 are fully
                    # masked -> skip them in scores, exp, mask and PV
                    q0 = qc * 512
                    j = kc - qc * 4
                    off = max(0, j) * P
                    QTa, QTb = QT[0:D, pr, :], QT[D:P, pr, :]
                    KTa, KTb = KT[0:D, pr, :], KT[D:P, pr, :]
                    sps = psE.tile([P, 1024], fp32, tag="sps", bufs=2, name="sps")
                    nc.tensor.matmul(
                        sps[:, off:512], KTa[:, kc * P:(kc + 1) * P],
                        QTa[:, q0 + off:q0 + 512], start=True, stop=True,
                        tile_position=(0, 0))
                    nc.tensor.matmul(
                        sps[:, 512 + off:1024], KTb[:, kc * P:(kc + 1) * P],
                        QTb[:, q0 + off:q0 + 512], start=True, stop=True,
                        tile_position=(64, 0))
                    sps2 = sps.rearrange("p (h f) -> p h f", h=2)
                    bias_t = mask_t if qc == 0 else mask2_t
                    nc.scalar.activation(et[:, kc % 2, :, off:],
                                         sps2[:, :, off:],
                                         AF.Exp, scale=scale,
                                         bias=bias_t[:, kc:kc + 1])
                    if j >= 0:  # zero the partial 128-wide triangle band;
                        # columns >= off+128 of this block are fully unmasked
                        csl = (cm[:, None, j, off:off + P] if qc == 0
                               else cm8[:, None, j, :])
                        nc.vector.tensor_mul(
                            et[:, kc % 2, :, off:off + P],
                            et[:, kc % 2, :, off:off + P],
                            csl.to_broadcast([P, 2, P]))
                    return off

                def pv16(pr, kc, nkc, off, et, ctxa, ctxb):
                    # first 512 q rows: plain bf16 matmul per key block
                    ha, hb = 2 * pr, 2 * pr + 1
                    nc.tensor.matmul(
                        ctxa[:, off:], Vt16_4[:, kc, ha, :],
                        et[:, kc % 2, 0, off:],
                        start=(kc == 0), stop=(kc == nkc - 1))
                    nc.tensor.matmul(
                        ctxb[:, off:], Vt16_4[:, kc, hb, :],
                        et[:, kc % 2, 1, off:],
                        start=(kc == 0), stop=(kc == nkc - 1))

                def pv_dr(pr, qc, kcp, off_pair, et, ctxa, ctxb):
                    # fp8 DoubleRow: both key blocks of the pair in one
                    # matmul per head (0.5 cycles/row)
                    ha, hb = 2 * pr, 2 * pr + 1
                    nkcp = 2 * (qc + 1)
                    nc.tensor.matmul(
                        ctxa[:, off_pair:],
                        Vt8_4[:, 2 * kcp:2 * kcp + 2, ha, :],
                        et[:, :, 0, off_pair:],
                        start=(kcp == 0), stop=(kcp == nkcp - 1),
                        perf_mode=DR)
                    nc.tensor.matmul(
                        ctxb[:, off_pair:],
                        Vt8_4[:, 2 * kcp:2 * kcp + 2, hb, :],
                        et[:, :, 1, off_pair:],
                        start=(kcp == 0), stop=(kcp == nkcp - 1),
                        perf_mode=DR)

                def tail(h, qc, ctx):
                    q0 = qc * 512
                    rcp = rcpp.tile([1, 512], fp32, tag="rcp", name="rcp")
                    nc.vector.reciprocal(rcp[0:1, :], ctx[0:1, :])
                    rb = rcpp.tile([VOFF + D, 512], fp32, tag="rb", name="rb")
                    nc.gpsimd.partition_broadcast(rb[:], rcp[0:1, :])
                    ctxn = cnp.tile([VOFF + D, 512], fp32, tag="cn", name="cn")
                    nc.vector.tensor_mul(ctxn[VOFF:, :], ctx[VOFF:VOFF + D, :],
                                         rb[VOFF:, :])
                    nc.sync.dma_start(
                        out[h * D:(h + 1) * D, q0:q0 + 512], ctxn[VOFF:, :])

                # ---------- schedule ----------
                # prologue block 0: x(sc0..3) -> xT, QT/KT dc0 sq0, V scp0..1
                for sc in range(4):
                    emit_xtr(early_xt[sc], sc, 0)
                    emit_xtr(early_xt[sc], sc, 1)
                for cl in proj_closures(wq_t, bq_t, QT, 0, 0):
                    cl()
                for cl in proj_closures(wk_t, bk_t, KT, 0, 0):
                    cl()
                for cl in v_closures(0) + v_closures(1):
                    cl()
                # Vt fixed columns (on GPSIMD so they don't block the first
                # xT evacuations in DVE's queue)
                nc.gpsimd.memset(Vt8_4[:, :, :, 0:1], 1.0)
                nc.gpsimd.memset(Vt8_4[:, :, :, 1:VOFF], 0.0)
                nc.gpsimd.memset(Vt16_4[:, :, :, 0:1], 1.0)
                nc.gpsimd.memset(Vt16_4[:, :, :, 1:VOFF], 0.0)

                # filler blocks 1..3 + C dc1 (+ dc0 later-sq), with markers
                for g in range(1, 4):
                    def blk(g=g):
                        loads, trs = [], []
                        boxes = {}
                        for sc in range(4 * g, 4 * g + 4):
                            boxes[sc] = []

                            def load(sc=sc):
                                boxes[sc].append(emit_xload(sc))

                            loads.append(load)
                            for hg in range(2):
                                def tr(sc=sc, hg=hg):
                                    emit_xtr(boxes[sc][0], sc, hg)
                                trs.append(tr)
                        # 2-deep DMA lookahead: L L t t L t t L t t t t
                        out_cl = [loads[0], loads[1], trs[0], trs[1],
                                  loads[2], trs[2], trs[3],
                                  loads[3], trs[4], trs[5], trs[6], trs[7]]
                        out_cl += proj_closures(wq_t, bq_t, QT, 0, g)
                        out_cl += proj_closures(wk_t, bk_t, KT, 0, g)
                        out_cl += v_closures(2 * g)
                        out_cl += v_closures(2 * g + 1)
                        return out_cl
                    add_fillers(blk())
                    set_marker(("blk", g))
                for sq in range(QC):
                    add_fillers(proj_closures(wk_t, bk_t, KT, 1, sq))
                for sq in (3, 2, 1, 0):
                    add_fillers(proj_closures(wq_t, bq_t, QT, 1, sq))
                    set_marker(("cdc1", sq))

                for pr in range(2):
                    qcs = list(range(QC)) if pr == 0 else list(range(QC))[::-1]
                    flat = [(qc, kc) for qc in qcs
                            for kc in range(4 * (qc + 1))]
                    ctxs = {}
                    ets = {}      # (qc, kcp) -> et pair tile
                    offs = {}     # (qc, kc) -> off

                    def ensure(qc):
                        if pr == 0:
                            if qc > 0:
                                drain_to(("blk", qc))
                        else:
                            drain_to(("cdc1", qc))

                    def start_unit(qc):
                        ensure(qc)
                        ctxs[qc] = (
                            psE.tile([VOFF + D, 512], fp32, tag="ctx", bufs=2, name="ctx"),
                            psE.tile([VOFF + D, 512], fp32, tag="ctx", bufs=2, name="ctx"))

                    def emit_scores(qc, kc):
                        kcp = kc // 2
                        if kc % 2 == 0:
                            if qc == 0:
                                et = et16p.tile([P, 2, 2, 512], bf16,
                                                tag="et16", name="et16")
                            else:
                                et = et8p.tile([P, 2, 2, 512], fp8,
                                               tag="et8", name="et8")
                                j_e = kc - qc * 4
                                if j_e >= 0:
                                    # odd member's fully-masked 128-band
                                    # (the DR moving starts at the even off)
                                    nc.gpsimd.memset(
                                        et[:, 1, :, j_e * P:(j_e + 1) * P],
                                        0.0)
                            ets[(qc, kcp)] = et
                        offs[(qc, kc)] = sc_exp(pr, qc, kc, ets[(qc, kcp)])

                    start_unit(flat[0][0])
                    for ahead in range(2):
                        qc, kc = flat[ahead]
                        emit_scores(qc, kc)
                    for i, (qc, kc) in enumerate(flat):
                        nkc = 4 * (qc + 1)
                        kcp = kc // 2
                        if qc == 0:
                            pv16(pr, kc, nkc, offs.pop((qc, kc)),
                                 ets[(qc, kcp)], ctxs[qc][0], ctxs[qc][1])
                            if kc % 2 == 1:
                                ets.pop((qc, kcp))
                        elif kc % 2 == 1:
                            off_pair = offs.pop((qc, kc - 1))
                            offs.pop((qc, kc))
                            pv_dr(pr, qc, kcp, off_pair,
                                  ets.pop((qc, kcp)), ctxs[qc][0], ctxs[qc][1])
                        pull(3 if pr == 0 else 1)
                        if i + 2 < len(flat):
                            q2, k2 = flat[i + 2]
                            if k2 == 0:
                                start_unit(q2)
                            emit_scores(q2, k2)
                        if kc == nkc - 1:
                            ca, cb = ctxs.pop(qc)
                            tail(2 * pr, qc, ca)
                            tail(2 * pr + 1, qc, cb)
                # drain any remaining fillers (shouldn't be many)
                while fillers:
                    pull(1)


def build():
    if "nc" not in _CACHE:
        nc = bacc.Bacc("TRN2", target_bir_lowering=False, debug=False,
                       num_devices=NCORES)
        _emit(nc)
        nc.compile()
        _CACHE["nc"] = nc
    return _CACHE["nc"]


def make_in_maps(hidden_states, attention_mask, Wq, bq, Wk, bk, Wv, bv):
    in_maps = []
    for c in range(NCORES):
        b, g = c // 4, c % 4
        sl = slice(g * HGD, (g + 1) * HGD)
        in_maps.append({
            "x": np.ascontiguousarray(hidden_states[b]),
            "wq": np.ascontiguousarray(Wq[:, sl]),
            "wk": np.ascontiguousarray(Wk[:, sl]),
            "wv": np.ascontiguousarray(Wv[:, sl]),
            "bq": np.ascontiguousarray(bq[sl]),
            "bk": np.ascontiguousarray(bk[sl]),
            "bv": np.ascontiguousarray(bv[sl]),
            "mask": np.ascontiguousarray(attention_mask[b, 0, 0, :]),
        })
    return in_maps


def kernel(hidden_states, attention_mask, Wq, bq, Wk, bk, Wv, bv, **run_kwargs):
    global LAST_RESULTS
    hidden_states = np.asarray(hidden_states, dtype=np.float32)
    attention_mask = np.asarray(attention_mask, dtype=np.float32)
    nc = build()
    in_maps = make_in_maps(
        hidden_states, attention_mask,
        np.asarray(Wq, np.float32), np.asarray(bq, np.float32),
        np.asarray(Wk, np.float32), np.asarray(bk, np.float32),
        np.asarray(Wv, np.float32), np.asarray(bv, np.float32))
    res = run_bass_kernel_spmd(nc, in_maps, core_ids=list(range(NCORES)),
                               **run_kwargs)
    LAST_RESULTS = res
    full = np.empty((B, S, H), dtype=np.float32)
    for c in range(NCORES):
        b, g = c // 4, c % 4
        full[b, :, g * HGD:(g + 1) * HGD] = res.results[c]["out"].T
    return full



# revision 20
# speedup vs baseline: 1.0145x; 1.0145x over previous
"""Causal self-attention (B=2, S=2048, H=1024, 16 heads) on 8 trn2 NeuronCores.

Sharding: core c handles batch b = c // 4 and head-group g = c % 4
(4 heads x d=64 = 256 output columns). Fully parallel, no collectives.

v3 pipeline (per core, all matmuls f32r = TF32-class):
  - x -> xT via f32r PE transposes (4 per PSUM bank, one [128,512] DVE evac)
  - QT/KT = W^T xT + b (d on partitions), V natural with [1 | V_h] interleave
  - scores^T per head PAIR packed into the PE array via tile_position
    (d=64 contraction fills half the rows); one exp over [128,1024] PSUM
  - causal diag masking: one half on DVE (0/1 mask mul), one on GPSIMD
  - PV: [1|V_h]^T @ expT accumulated in PSUM -> [65, 512]; row 0 = softmax
    denominator; tail = reciprocal + SBUF DMA-broadcast + one DVE multiply;
    output stored TRANSPOSED [256, 2048] per core, host transposes back
  - emission: software-pipelined score stream (2 ahead of PV) with the
    remaining projection/V/transpose work woven in as fine-grained fillers
    so ACT's exp pipeline saturates from ~10us onward
"""

from collections import deque

import numpy as np

import concourse.bacc as bacc
import concourse.mybir as mybir
from concourse.tile import TileContext
from concourse.bass_utils import run_bass_kernel_spmd
from concourse.masks import make_identity

B, S, H, NH, D = 2, 2048, 1024, 16, 64
P = 128
NCORES = 8
NHL = NH // 4            # 4 heads per core
HGD = NHL * D            # 256 output cols per core
HC = H // P              # 8 contraction chunks
SC = S // P              # 16 sequence chunks of 128
QC = S // 512            # 4 query chunks of 512
KC = S // P              # 16 key chunks of 128
DC = HGD // P            # 2 partition chunks of QT/KT

fp32 = mybir.dt.float32
f32r = mybir.dt.float32r
bf16 = mybir.dt.bfloat16
fp8 = mybir.dt.float8e4
AF = mybir.ActivationFunctionType
ALU = mybir.AluOpType
DR = mybir.MatmulPerfMode.DoubleRow

_CACHE = {}
LAST_RESULTS = None


def _emit(nc):
    x = nc.declare_dram_parameter("x", [S, H], f32r, isOutput=False)
    wq = nc.declare_dram_parameter("wq", [H, HGD], f32r, isOutput=False)
    wk = nc.declare_dram_parameter("wk", [H, HGD], f32r, isOutput=False)
    wv = nc.declare_dram_parameter("wv", [H, HGD], f32r, isOutput=False)
    bq = nc.declare_dram_parameter("bq", [HGD], fp32, isOutput=False)
    bk = nc.declare_dram_parameter("bk", [HGD], fp32, isOutput=False)
    bv = nc.declare_dram_parameter("bv", [HGD], fp32, isOutput=False)
    mask = nc.declare_dram_parameter("mask", [S], fp32, isOutput=False)
    # transposed output: host does the final [HGD, S] -> [S, HGD] transpose
    out = nc.declare_dram_parameter("out", [HGD, S], fp32, isOutput=True)

    scale = float(1.0 / np.sqrt(np.float32(D)))

    with TileContext(nc) as tc:
        with tc.tile_pool(name="const", bufs=1) as const, \
             tc.tile_pool(name="big", bufs=1) as big:

            # ---- big tiles ----
            xT = big.tile([P, HC, S], f32r, tag="xT")
            # f32r Q/K only for the accuracy-critical first 512 q rows /
            # first 4 key blocks; everything else reads the fp8 copies
            QT = big.tile([P, DC, 512], f32r, tag="QT")
            KT = big.tile([P, DC, 512], f32r, tag="KT")
            VS, VOFF = 128, 64  # per-head [1 | zeros*63 | V] stationary layout
            # fp8 V (all key blocks) for DoubleRow PV on q rows >= 512;
            # bf16 V (key blocks 0..3) for the accuracy-critical first 512 rows
            Vt8 = big.tile([P, SC, NHL * VS], fp8, tag="Vt8")
            Vt8_4 = Vt8.rearrange("p sc (h c) -> p sc h c", c=VS)
            Vt16 = big.tile([P, 4, NHL * VS], bf16, tag="Vt16")
            Vt16_4 = Vt16.rearrange("p sc (h c) -> p sc h c", c=VS)
            # fp8 Q/K in DoubleRow layout for q rows >= 512 scores:
            # head h lives on partitions 32h..32h+32, free = (d-half t, s):
            # Q8[32h+p, t, s] = Q[head h, d=32t+p, s]
            Q8 = big.tile([P, 2, S], fp8, tag="Q8")
            K8 = big.tile([P, 2, S], fp8, tag="K8")

            with tc.tile_pool(name="xin", bufs=4) as xin, \
                 tc.tile_pool(name="et8", bufs=3) as et8p, \
                 tc.tile_pool(name="et16", bufs=3) as et16p, \
                 tc.tile_pool(name="stg", bufs=3) as stgp, \
                 tc.tile_pool(name="rcp", bufs=2) as rcpp, \
                 tc.tile_pool(name="cnorm", bufs=2) as cnp, \
                 tc.tile_pool(name="psW", bufs=2, space="PSUM") as psW, \
                 tc.tile_pool(name="psE", bufs=1, space="PSUM") as psE:

                # ---------- early loads: x first (PE depends on it) ----------
                def emit_xload(sc):
                    xt = xin.tile([P, H], f32r, tag="xt", name="xt")
                    nc.sync.dma_start(xt[:], x[sc * P:(sc + 1) * P, :])
                    return xt

                xt0 = xin.tile([P, H], f32r, tag="xt", name="xt")
                nc.sync.dma_start(xt0[:, 0:512], x[0:P, 0:512])
                nc.sync.dma_start(xt0[:, 512:H], x[0:P, 512:H])
                early_xt = [xt0] + [emit_xload(sc) for sc in range(1, 4)]

                ident_f = const.tile([P, P], fp32, tag="identf")
                make_identity(nc, ident_f)
                ident_r = const.tile([P, P], f32r, tag="identr")
                nc.vector.tensor_copy(ident_r[:], ident_f[:])
                # 0/1 causal masks: cm[:, j, f] = (f - p >= -j*128)
                cm = const.tile([P, 4, 512], mybir.dt.bfloat16, tag="cmask")
                nc.gpsimd.memset(cm[:], 1.0)
                for j in range(4):
                    nc.gpsimd.affine_select(
                        out=cm[:, j, :], in_=cm[:, j, :],
                        compare_op=ALU.is_ge, fill=0.0,
                        base=-j * P, pattern=[[1, 512]], channel_multiplier=-1)
                # only the 128-wide partial band [j*128, j*128+128) is ever
                # applied; keep an fp8 copy of those bands for the fp8 path
                cm8 = const.tile([P, 4, P], fp8, tag="cmask8")
                for j in range(4):
                    nc.gpsimd.tensor_copy(cm8[:, j, :], cm[:, j, j * P:(j + 1) * P])

                # weights / biases / mask (needed ~15us in)
                wq_t = big.tile([P, HC, HGD], f32r, tag="wq")
                wk_t = big.tile([P, HC, HGD], f32r, tag="wk")
                wv_t = big.tile([P, HC, HGD], f32r, tag="wv")
                nc.sync.dma_start(wq_t[:], wq[:].rearrange("(hc p) n -> p hc n", p=P))
                nc.sync.dma_start(wk_t[:], wk[:].rearrange("(hc p) n -> p hc n", p=P))
                nc.sync.dma_start(wv_t[:], wv[:].rearrange("(hc p) n -> p hc n", p=P))
                bq_t = const.tile([P, DC], fp32, tag="bq")
                bk_t = const.tile([P, DC], fp32, tag="bk")
                nc.sync.dma_start(bq_t[:], bq[:].rearrange("(dc p) -> p dc", p=P))
                nc.sync.dma_start(bk_t[:], bk[:].rearrange("(dc p) -> p dc", p=P))
                bv_b = const.tile([P, HGD], fp32, tag="bv")
                nc.gpsimd.dma_start(bv_b[:], bv[None, :].to_broadcast([P, HGD]))
                bv4 = bv_b.rearrange("p (h c) -> p h c", c=D)
                mask_t = const.tile([P, KC], fp32, tag="mask")
                nc.sync.dma_start(mask_t[:], mask[:].rearrange("(kc p) -> p kc", p=P))
                # fp8-path exp bias: mask - 3. Softmax is shift-invariant (the
                # ones-column denominator absorbs e^-3), and the shift moves
                # fp8e4 overflow (448) from ~6 sigma scores out to ~9 sigma.
                mask2_t = const.tile([P, KC], fp32, tag="mask2")
                neg3_c = const.tile([P, 1], fp32, tag="neg3")
                nc.vector.memset(neg3_c[:], -3.0)
                nc.vector.tensor_scalar_add(mask2_t[:], mask_t[:],
                                            neg3_c[:, 0:1])

                zero_c = const.tile([P, 1], fp32, tag="zero")
                nc.vector.memset(zero_c[:], 0.0)
                ones_c = const.tile([P, 1], fp32, tag="ones")
                nc.vector.memset(ones_c[:], 1.0)

                def emit_xtr(xt, sc, hg):  # 4 transposes + 1 evac
                    tp = psW.tile([P, 512], f32r, tag="pp", name="tp")
                    for j in range(4):
                        hc = hg * 4 + j
                        nc.tensor.matmul(
                            tp[:, j * P:(j + 1) * P],
                            xt[:, hc * P:(hc + 1) * P], ident_r[:],
                            is_transpose=True, start=(j == 0), stop=(j == 3),
                            skip_group_check=True)
                    nc.vector.tensor_copy(
                        xT[:, hg * 4:(hg + 1) * 4, sc * P:(sc + 1) * P],
                        tp.rearrange("p (j c) -> p j c", c=P))

                def proj_closures(W, bias_t, OUTF, OUT8, dc, sq):
                    # 8 hc matmuls accumulate in PSUM; evac with bias to fp8
                    # staging (DoubleRow layout via 4 remap DMAs), plus an
                    # f32r copy of the sq==0 chunk for the qc0 path
                    pp = [None]
                    stg = [None]

                    def mk(hc):
                        def go():
                            if hc == 0:
                                pp[0] = psW.tile([P, 512], fp32, tag="pp", name="pp")
                            nc.tensor.matmul(
                                pp[0][:], W[:, hc, dc * P:(dc + 1) * P],
                                xT[:, hc, sq * 512:(sq + 1) * 512],
                                start=(hc == 0), stop=(hc == HC - 1))
                            if hc == HC - 1:
                                if sq == 0:
                                    nc.vector.tensor_scalar_add(
                                        OUTF[:, dc, :],
                                        pp[0][:], bias_t[:, dc:dc + 1])
                                stg[0] = stgp.tile([P, 512], fp8, tag="stg",
                                                   name="stg")
                                nc.vector.tensor_scalar_add(
                                    stg[0][:], pp[0][:], bias_t[:, dc:dc + 1])
                        return go

                    def mk_dma(hh, t):
                        def go():
                            hg = 2 * dc + hh
                            nc.sync.dma_start(
                                OUT8[32 * hg:32 * hg + 32, t,
                                     sq * 512:(sq + 1) * 512],
                                stg[0][64 * hh + 32 * t:64 * hh + 32 * t + 32,
                                       :])
                        return go
                    return ([mk(hc) for hc in range(HC)] +
                            [mk_dma(hh, t) for hh in range(2)
                             for t in range(2)])

                def v_closures(scp):  # V for sc pair -> one [128,512] bank
                    pp = [None]

                    def mk(half, hc):
                        def go():
                            if half == 0 and hc == 0:
                                pp[0] = psW.tile([P, 512], fp32, tag="pp", name="pp")
                            sc = scp * 2 + half
                            nc.tensor.matmul(
                                pp[0][:, half * HGD:(half + 1) * HGD],
                                xT[:, hc, sc * P:(sc + 1) * P], wv_t[:, hc, :],
                                start=(half == 0 and hc == 0),
                                stop=(hc == HC - 1), skip_group_check=True)
                            if half == 1 and hc == HC - 1:
                                pp2 = pp[0].rearrange("p (s h c) -> p s h c",
                                                      s=2, c=D)
                                bvb = bv4[:, None, :, :].to_broadcast(
                                    [P, 2, NHL, D])
                                nc.vector.tensor_tensor(
                                    Vt8_4[:, scp * 2:scp * 2 + 2, :,
                                          VOFF:VOFF + D],
                                    pp2, bvb, ALU.add)
                                if scp < 2:
                                    nc.vector.tensor_tensor(
                                        Vt16_4[:, scp * 2:scp * 2 + 2, :,
                                               VOFF:VOFF + D],
                                        pp2, bvb, ALU.add)
                        return go
                    return [mk(h, hc) for h in range(2) for hc in range(HC)]

                # ---------- filler queue with dependency markers ----------
                fillers = deque()
                markers = {}
                done = [0]

                def pull(n):
                    for _ in range(n):
                        if not fillers:
                            return
                        fillers.popleft()()
                        done[0] += 1

                def drain_to(marker):
                    tgt = markers.get(marker, 0)
                    while done[0] < tgt:
                        fillers.popleft()()
                        done[0] += 1

                def add_fillers(closures):
                    fillers.extend(closures)

                def set_marker(name):
                    markers[name] = done[0] + len(fillers)

                # ---------- attention ----------
                # et tiles hold an exp'd score PAIR of key blocks:
                # [P, kc%2, head-of-pair, 512q]. fp8 for q rows >= 512
                # (consumed by DoubleRow PV over the kc pair), bf16 for the
                # accuracy-critical first 512 rows (plain per-kc PV).
                def sc_exp(pr, qc, kc, et):
                    # diagonal tiles (j >= 0): columns f < j*128 are fully
                    # masked -> skip them in scores, exp, mask and PV
                    q0 = qc * 512
                    j = kc - qc * 4
                    off = max(0, j) * P
                    sps = psE.tile([P, 1024], fp32, tag="sps", bufs=2, name="sps")
                    if qc == 0:
                        QTa, QTb = QT[0:D, pr, :], QT[D:P, pr, :]
                        KTa, KTb = KT[0:D, pr, :], KT[D:P, pr, :]
                        nc.tensor.matmul(
                            sps[:, off:512], KTa[:, kc * P:(kc + 1) * P],
                            QTa[:, q0 + off:q0 + 512], start=True, stop=True,
                            tile_position=(0, 0))
                        nc.tensor.matmul(
                            sps[:, 512 + off:1024], KTb[:, kc * P:(kc + 1) * P],
                            QTb[:, q0 + off:q0 + 512], start=True, stop=True,
                            tile_position=(64, 0))
                    else:
                        # fp8 DoubleRow: d=64 contraction as two 32-row
                        # k-tiles, 0.5 cycles/row
                        for hh in range(2):
                            b0 = 32 * (2 * pr + hh)
                            nc.tensor.matmul(
                                sps[:, hh * 512 + off:(hh + 1) * 512],
                                K8[b0:b0 + 32, :, kc * P:(kc + 1) * P],
                                Q8[b0:b0 + 32, :, q0 + off:q0 + 512],
                                start=True, stop=True, perf_mode=DR,
                                tile_position=(b0, 0))
                    sps2 = sps.rearrange("p (h f) -> p h f", h=2)
                    bias_t = mask_t if qc == 0 else mask2_t
                    nc.scalar.activation(et[:, kc % 2, :, off:],
                                         sps2[:, :, off:],
                                         AF.Exp, scale=scale,
                                         bias=bias_t[:, kc:kc + 1])
                    if j >= 0:  # zero the partial 128-wide triangle band;
                        # columns >= off+128 of this block are fully unmasked
                        csl = (cm[:, None, j, off:off + P] if qc == 0
                               else cm8[:, None, j, :])
                        nc.vector.tensor_mul(
                            et[:, kc % 2, :, off:off + P],
                            et[:, kc % 2, :, off:off + P],
                            csl.to_broadcast([P, 2, P]))
                    return off

                def pv16(pr, kc, nkc, off, et, ctxa, ctxb):
                    # first 512 q rows: plain bf16 matmul per key block
                    ha, hb = 2 * pr, 2 * pr + 1
                    nc.tensor.matmul(
                        ctxa[:, off:], Vt16_4[:, kc, ha, :],
                        et[:, kc % 2, 0, off:],
                        start=(kc == 0), stop=(kc == nkc - 1))
                    nc.tensor.matmul(
                        ctxb[:, off:], Vt16_4[:, kc, hb, :],
                        et[:, kc % 2, 1, off:],
                        start=(kc == 0), stop=(kc == nkc - 1))

                def pv_dr(pr, qc, kcp, off_pair, et, ctxa, ctxb):
                    # fp8 DoubleRow: both key blocks of the pair in one
                    # matmul per head (0.5 cycles/row)
                    ha, hb = 2 * pr, 2 * pr + 1
                    nkcp = 2 * (qc + 1)
                    nc.tensor.matmul(
                        ctxa[:, off_pair:],
                        Vt8_4[:, 2 * kcp:2 * kcp + 2, ha, :],
                        et[:, :, 0, off_pair:],
                        start=(kcp == 0), stop=(kcp == nkcp - 1),
                        perf_mode=DR)
                    nc.tensor.matmul(
                        ctxb[:, off_pair:],
                        Vt8_4[:, 2 * kcp:2 * kcp + 2, hb, :],
                        et[:, :, 1, off_pair:],
                        start=(kcp == 0), stop=(kcp == nkcp - 1),
                        perf_mode=DR)

                def tail(h, qc, ctx):
                    q0 = qc * 512
                    rcp = rcpp.tile([1, 512], fp32, tag="rcp", name="rcp")
                    nc.vector.reciprocal(rcp[0:1, :], ctx[0:1, :])
                    rb = rcpp.tile([VOFF + D, 512], fp32, tag="rb", name="rb")
                    nc.gpsimd.partition_broadcast(rb[:], rcp[0:1, :])
                    ctxn = cnp.tile([VOFF + D, 512], fp32, tag="cn", name="cn")
                    nc.vector.tensor_mul(ctxn[VOFF:, :], ctx[VOFF:VOFF + D, :],
                                         rb[VOFF:, :])
                    nc.sync.dma_start(
                        out[h * D:(h + 1) * D, q0:q0 + 512], ctxn[VOFF:, :])

                # ---------- schedule ----------
                # prologue block 0: x(sc0..3) -> xT, QT/KT dc0 sq0, V scp0..1
                for sc in range(4):
                    emit_xtr(early_xt[sc], sc, 0)
                    emit_xtr(early_xt[sc], sc, 1)
                for cl in proj_closures(wq_t, bq_t, QT, Q8, 0, 0):
                    cl()
                for cl in proj_closures(wk_t, bk_t, KT, K8, 0, 0):
                    cl()
                for cl in v_closures(0) + v_closures(1):
                    cl()
                # Vt fixed columns (on GPSIMD so they don't block the first
                # xT evacuations in DVE's queue)
                nc.gpsimd.memset(Vt8_4[:, :, :, 0:1], 1.0)
                nc.gpsimd.memset(Vt8_4[:, :, :, 1:VOFF], 0.0)
                nc.gpsimd.memset(Vt16_4[:, :, :, 0:1], 1.0)
                nc.gpsimd.memset(Vt16_4[:, :, :, 1:VOFF], 0.0)

                # filler blocks 1..3 + C dc1 (+ dc0 later-sq), with markers
                for g in range(1, 4):
                    def blk(g=g):
                        loads, trs = [], []
                        boxes = {}
                        for sc in range(4 * g, 4 * g + 4):
                            boxes[sc] = []

                            def load(sc=sc):
                                boxes[sc].append(emit_xload(sc))

                            loads.append(load)
                            for hg in range(2):
                                def tr(sc=sc, hg=hg):
                                    emit_xtr(boxes[sc][0], sc, hg)
                                trs.append(tr)
                        # 2-deep DMA lookahead: L L t t L t t L t t t t
                        out_cl = [loads[0], loads[1], trs[0], trs[1],
                                  loads[2], trs[2], trs[3],
                                  loads[3], trs[4], trs[5], trs[6], trs[7]]
                        out_cl += proj_closures(wq_t, bq_t, QT, Q8, 0, g)
                        out_cl += proj_closures(wk_t, bk_t, KT, K8, 0, g)
                        out_cl += v_closures(2 * g)
                        out_cl += v_closures(2 * g + 1)
                        return out_cl
                    add_fillers(blk())
                    set_marker(("blk", g))
                for sq in range(QC):
                    add_fillers(proj_closures(wk_t, bk_t, KT, K8, 1, sq))
                for sq in (3, 2, 1, 0):
                    add_fillers(proj_closures(wq_t, bq_t, QT, Q8, 1, sq))
                    set_marker(("cdc1", sq))

                for pr in range(2):
                    qcs = list(range(QC)) if pr == 0 else list(range(QC))[::-1]
                    flat = [(qc, kc) for qc in qcs
                            for kc in range(4 * (qc + 1))]
                    ctxs = {}
                    ets = {}      # (qc, kcp) -> et pair tile
                    offs = {}     # (qc, kc) -> off

                    def ensure(qc):
                        if pr == 0:
                            if qc > 0:
                                drain_to(("blk", qc))
                        else:
                            drain_to(("cdc1", qc))

                    def start_unit(qc):
                        ensure(qc)
                        ctxs[qc] = (
                            psE.tile([VOFF + D, 512], fp32, tag="ctx", bufs=2, name="ctx"),
                            psE.tile([VOFF + D, 512], fp32, tag="ctx", bufs=2, name="ctx"))

                    def emit_scores(qc, kc):
                        kcp = kc // 2
                        if kc % 2 == 0:
                            if qc == 0:
                                et = et16p.tile([P, 2, 2, 512], bf16,
                                                tag="et16", name="et16")
                            else:
                                et = et8p.tile([P, 2, 2, 512], fp8,
                                               tag="et8", name="et8")
                                j_e = kc - qc * 4
                                if j_e >= 0:
                                    # odd member's fully-masked 128-band
                                    # (the DR moving starts at the even off)
                                    nc.gpsimd.memset(
                                        et[:, 1, :, j_e * P:(j_e + 1) * P],
                                        0.0)
                            ets[(qc, kcp)] = et
                        offs[(qc, kc)] = sc_exp(pr, qc, kc, ets[(qc, kcp)])

                    start_unit(flat[0][0])
                    for ahead in range(2):
                        qc, kc = flat[ahead]
                        emit_scores(qc, kc)
                    for i, (qc, kc) in enumerate(flat):
                        nkc = 4 * (qc + 1)
                        kcp = kc // 2
                        if qc == 0:
                            pv16(pr, kc, nkc, offs.pop((qc, kc)),
                                 ets[(qc, kcp)], ctxs[qc][0], ctxs[qc][1])
                            if kc % 2 == 1:
                                ets.pop((qc, kcp))
                        elif kc % 2 == 1:
                            off_pair = offs.pop((qc, kc - 1))
                            offs.pop((qc, kc))
                            pv_dr(pr, qc, kcp, off_pair,
                                  ets.pop((qc, kcp)), ctxs[qc][0], ctxs[qc][1])
                        pull(3 if pr == 0 else 1)
                        if i + 2 < len(flat):
                            q2, k2 = flat[i + 2]
                            if k2 == 0:
                                start_unit(q2)
                            emit_scores(q2, k2)
                        if kc == nkc - 1:
                            ca, cb = ctxs.pop(qc)
                            tail(2 * pr, qc, ca)
                            tail(2 * pr + 1, qc, cb)
                # drain any remaining fillers (shouldn't be many)
                while fillers:
                    pull(1)


def build():
    if "nc" not in _CACHE:
        nc = bacc.Bacc("TRN2", target_bir_lowering=False, debug=False,
                       num_devices=NCORES)
        _emit(nc)
        nc.compile()
        _CACHE["nc"] = nc
    return _CACHE["nc"]


def make_in_maps(hidden_states, attention_mask, Wq, bq, Wk, bk, Wv, bv):
    in_maps = []
    for c in range(NCORES):
        b, g = c // 4, c % 4
        sl = slice(g * HGD, (g + 1) * HGD)
        in_maps.append({
            "x": np.ascontiguousarray(hidden_states[b]),
            "wq": np.ascontiguousarray(Wq[:, sl]),
            "wk": np.ascontiguousarray(Wk[:, sl]),
            "wv": np.ascontiguousarray(Wv[:, sl]),
            "bq": np.ascontiguousarray(bq[sl]),
            "bk": np.ascontiguousarray(bk[sl]),
            "bv": np.ascontiguousarray(bv[sl]),
            "mask": np.ascontiguousarray(attention_mask[b, 0, 0, :]),
        })
    return in_maps


def kernel(hidden_states, attention_mask, Wq, bq, Wk, bk, Wv, bv, **run_kwargs):
    global LAST_RESULTS
    hidden_states = np.asarray(hidden_states, dtype=np.float32)
    attention_mask = np.asarray(attention_mask, dtype=np.float32)
    nc = build()
    in_maps = make_in_maps(
        hidden_states, attention_mask,
        np.asarray(Wq, np.float32), np.asarray(bq, np.float32),
        np.asarray(Wk, np.float32), np.asarray(bk, np.float32),
        np.asarray(Wv, np.float32), np.asarray(bv, np.float32))
    res = run_bass_kernel_spmd(nc, in_maps, core_ids=list(range(NCORES)),
                               **run_kwargs)
    LAST_RESULTS = res
    full = np.empty((B, S, H), dtype=np.float32)
    for c in range(NCORES):
        b, g = c // 4, c % 4
        full[b, :, g * HGD:(g + 1) * HGD] = res.results[c]["out"].T
    return full



# revision 39
# speedup vs baseline: 1.0765x; 1.0611x over previous
"""Causal self-attention (B=2, S=2048, H=1024, 16 heads) on 8 trn2 NeuronCores.

Sharding: core c handles batch b = c // 4 and head-group g = c % 4
(4 heads x d=64 = 256 output columns). Fully parallel, no collectives.

v3 pipeline (per core, all matmuls f32r = TF32-class):
  - x -> xT via f32r PE transposes (4 per PSUM bank, one [128,512] DVE evac)
  - QT/KT = W^T xT + b (d on partitions), V natural with [1 | V_h] interleave
  - scores^T per head PAIR packed into the PE array via tile_position
    (d=64 contraction fills half the rows); one exp over [128,1024] PSUM
  - causal diag masking: one half on DVE (0/1 mask mul), one on GPSIMD
  - PV: [1|V_h]^T @ expT accumulated in PSUM -> [65, 512]; row 0 = softmax
    denominator; tail = reciprocal + SBUF DMA-broadcast + one DVE multiply;
    output stored TRANSPOSED [256, 2048] per core, host transposes back
  - emission: software-pipelined score stream (2 ahead of PV) with the
    remaining projection/V/transpose work woven in as fine-grained fillers
    so ACT's exp pipeline saturates from ~10us onward
"""

from collections import deque

import ml_dtypes
import numpy as np

import concourse.bacc as bacc
import concourse.mybir as mybir
from concourse.tile import TileContext
from concourse.bass_utils import run_bass_kernel_spmd
from concourse.masks import make_identity

B, S, H, NH, D = 2, 2048, 1024, 16, 64
P = 128
NCORES = 8
NHL = NH // 4            # 4 heads per core
HGD = NHL * D            # 256 output cols per core
HC = H // P              # 8 contraction chunks
SC = S // P              # 16 sequence chunks of 128
QC = S // 512            # 4 query chunks of 512
KC = S // P              # 16 key chunks of 128
DC = HGD // P            # 2 partition chunks of QT/KT

fp32 = mybir.dt.float32
f32r = mybir.dt.float32r
bf16 = mybir.dt.bfloat16
fp8 = mybir.dt.float8e4
AF = mybir.ActivationFunctionType
ALU = mybir.AluOpType
DR = mybir.MatmulPerfMode.DoubleRow

_CACHE = {}
LAST_RESULTS = None


def _emit(nc):
    x = nc.declare_dram_parameter("x", [S, H], bf16, isOutput=False)
    wq = nc.declare_dram_parameter("wq", [H, HGD], bf16, isOutput=False)
    wk = nc.declare_dram_parameter("wk", [H, HGD], bf16, isOutput=False)
    wv = nc.declare_dram_parameter("wv", [H, HGD], bf16, isOutput=False)
    bq = nc.declare_dram_parameter("bq", [HGD], fp32, isOutput=False)
    bk = nc.declare_dram_parameter("bk", [HGD], fp32, isOutput=False)
    bv = nc.declare_dram_parameter("bv", [HGD], fp32, isOutput=False)
    mask = nc.declare_dram_parameter("mask", [S], fp32, isOutput=False)
    # transposed output: host does the final [HGD, S] -> [S, HGD] transpose
    out = nc.declare_dram_parameter("out", [HGD, S], fp32, isOutput=True)

    scale = float(1.0 / np.sqrt(np.float32(D)))

    with TileContext(nc) as tc:
        with tc.tile_pool(name="const", bufs=1) as const, \
             tc.tile_pool(name="big", bufs=1) as big:

            # ---- big tiles ----
            xT = big.tile([P, HC, S], bf16, tag="xT")
            # f32r Q/K only for the accuracy-critical first 512 q rows /
            # first 4 key blocks; everything else reads the fp8 copies
            QT = big.tile([P, DC, 512], bf16, tag="QT")
            KT = big.tile([P, DC, 512], bf16, tag="KT")
            VS, VOFF = 128, 64  # per-head [1 | zeros*63 | V] stationary layout
            # fp8 V (all key blocks) for DoubleRow PV on q rows >= 512;
            # bf16 V (key blocks 0..3) for the accuracy-critical first 512 rows
            Vt8 = big.tile([P, SC, NHL * VS], fp8, tag="Vt8")
            Vt8_4 = Vt8.rearrange("p sc (h c) -> p sc h c", c=VS)
            Vt16 = big.tile([P, 4, NHL * VS], bf16, tag="Vt16")
            Vt16_4 = Vt16.rearrange("p sc (h c) -> p sc h c", c=VS)
            # fp8 Q/K in DoubleRow layout for q rows >= 512 scores:
            # head h lives on partitions 32h..32h+32, free = (d-half t, s):
            # Q8[32h+p, t, s] = Q[head h, d=32t+p, s]
            Q8 = big.tile([P, 2, S], fp8, tag="Q8")
            K8 = big.tile([P, 2, S], fp8, tag="K8")

            with tc.tile_pool(name="xin", bufs=4) as xin, \
                 tc.tile_pool(name="et8", bufs=3) as et8p, \
                 tc.tile_pool(name="et16", bufs=3) as et16p, \
                 tc.tile_pool(name="stg", bufs=3) as stgp, \
                 tc.tile_pool(name="rcp", bufs=2) as rcpp, \
                 tc.tile_pool(name="cnorm", bufs=2) as cnp, \
                 tc.tile_pool(name="psW", bufs=2, space="PSUM") as psW, \
                 tc.tile_pool(name="psE", bufs=1, space="PSUM") as psE:

                # ---------- early loads: x first (PE depends on it) ----------
                def emit_xload(sc):
                    xt = xin.tile([P, H], bf16, tag="xt", name="xt")
                    nc.sync.dma_start(xt[:], x[sc * P:(sc + 1) * P, :])
                    return xt

                xt0 = xin.tile([P, H], bf16, tag="xt", name="xt")
                nc.sync.dma_start(xt0[:, 0:512], x[0:P, 0:512])
                nc.sync.dma_start(xt0[:, 512:H], x[0:P, 512:H])
                early_xt = [xt0] + [emit_xload(sc) for sc in range(1, 4)]

                ident_f = const.tile([P, P], fp32, tag="identf")
                make_identity(nc, ident_f)
                ident_r = const.tile([P, P], bf16, tag="identr")
                nc.vector.tensor_copy(ident_r[:], ident_f[:])
                # dummy exp: pulls the 1.28us ACT table load into the idle
                # prologue instead of the first real score exp
                warm = const.tile([1, 1], fp32, tag="warm")
                nc.scalar.activation(warm[0:1, 0:1], ident_f[0:1, 0:1], AF.Exp)
                # 0/1 causal masks: cm[:, j, f] = (f - p >= -j*128)
                cm = const.tile([P, 4, 512], mybir.dt.bfloat16, tag="cmask")
                nc.gpsimd.memset(cm[:], 1.0)
                for j in range(4):
                    nc.gpsimd.affine_select(
                        out=cm[:, j, :], in_=cm[:, j, :],
                        compare_op=ALU.is_ge, fill=0.0,
                        base=-j * P, pattern=[[1, 512]], channel_multiplier=-1)
                # only the 128-wide partial band [j*128, j*128+128) is ever
                # applied; keep an fp8 copy of those bands for the fp8 path
                cm8 = const.tile([P, 4, P], fp8, tag="cmask8")
                for j in range(4):
                    nc.gpsimd.tensor_copy(cm8[:, j, :], cm[:, j, j * P:(j + 1) * P])

                # small bias/mask DMAs first: mask_t gates the very first
                # exp, so it must not queue behind 8.7us of W loads
                bq_t = const.tile([P, DC], fp32, tag="bq")
                bk_t = const.tile([P, DC], fp32, tag="bk")
                nc.sync.dma_start(bq_t[:], bq[:].rearrange("(dc p) -> p dc", p=P))
                nc.sync.dma_start(bk_t[:], bk[:].rearrange("(dc p) -> p dc", p=P))
                mask_t = const.tile([P, KC], fp32, tag="mask")
                nc.sync.dma_start(mask_t[:], mask[:].rearrange("(kc p) -> p kc", p=P))
                bv_b = const.tile([P, HGD], fp32, tag="bv")
                nc.gpsimd.dma_start(bv_b[:], bv[None, :].to_broadcast([P, HGD]))
                bv4 = bv_b.rearrange("p (h c) -> p h c", c=D)
                wq_t = big.tile([P, HC, HGD], bf16, tag="wq")
                wk_t = big.tile([P, HC, HGD], bf16, tag="wk")
                wv_t = big.tile([P, HC, HGD], bf16, tag="wv")
                # W loads on the ACT engine's DGE queue so the x loads
                # (sync queue) are not stuck behind 8.7us of weight traffic
                nc.sync.dma_start(wq_t[:], wq[:].rearrange("(hc p) n -> p hc n", p=P))
                nc.sync.dma_start(wk_t[:], wk[:].rearrange("(hc p) n -> p hc n", p=P))
                nc.sync.dma_start(wv_t[:], wv[:].rearrange("(hc p) n -> p hc n", p=P))
                # fp8-path exp bias: mask - 3. Softmax is shift-invariant (the
                # ones-column denominator absorbs e^-3), and the shift moves
                # fp8e4 overflow (448) from ~6 sigma scores out to ~9 sigma.
                mask2_t = const.tile([P, KC], fp32, tag="mask2")
                neg3_c = const.tile([P, 1], fp32, tag="neg3")
                nc.vector.memset(neg3_c[:], -3.0)
                nc.vector.tensor_scalar_add(mask2_t[:], mask_t[:],
                                            neg3_c[:, 0:1])

                zero_c = const.tile([P, 1], fp32, tag="zero")
                nc.vector.memset(zero_c[:], 0.0)
                ones_c = const.tile([P, 1], fp32, tag="ones")
                nc.vector.memset(ones_c[:], 1.0)

                def emit_xtr(xt, sc, hg):  # 4 transposes + 1 evac
                    tp = psW.tile([P, 512], bf16, tag="pp", name="tp")
                    for j in range(4):
                        hc = hg * 4 + j
                        nc.tensor.matmul(
                            tp[:, j * P:(j + 1) * P],
                            xt[:, hc * P:(hc + 1) * P], ident_r[:],
                            is_transpose=True, start=(j == 0), stop=(j == 3),
                            skip_group_check=True)
                    nc.vector.tensor_copy(
                        xT[:, hg * 4:(hg + 1) * 4, sc * P:(sc + 1) * P],
                        tp.rearrange("p (j c) -> p j c", c=P))

                def proj_closures(W, bias_t, OUTF, OUT8, dc, sq):
                    # 8 hc matmuls accumulate in PSUM; evac with bias to fp8
                    # staging (DoubleRow layout via 4 remap DMAs), plus an
                    # f32r copy of the sq==0 chunk for the qc0 path
                    pp = [None]
                    stg = [None]

                    def mk(hc):
                        def go():
                            if hc == 0:
                                pp[0] = psW.tile([P, 512], fp32, tag="pp", name="pp")
                            nc.tensor.matmul(
                                pp[0][:], W[:, hc, dc * P:(dc + 1) * P],
                                xT[:, hc, sq * 512:(sq + 1) * 512],
                                start=(hc == 0), stop=(hc == HC - 1))
                            if hc == HC - 1:
                                if sq == 0:
                                    nc.vector.tensor_scalar_add(
                                        OUTF[:, dc, :],
                                        pp[0][:], bias_t[:, dc:dc + 1])
                                stg[0] = stgp.tile([P, 512], fp8, tag="stg",
                                                   name="stg")
                                nc.vector.tensor_scalar_add(
                                    stg[0][:], pp[0][:], bias_t[:, dc:dc + 1])
                        return go

                    def mk_dma(hh, t):
                        def go():
                            hg = 2 * dc + hh
                            nc.sync.dma_start(
                                OUT8[32 * hg:32 * hg + 32, t,
                                     sq * 512:(sq + 1) * 512],
                                stg[0][64 * hh + 32 * t:64 * hh + 32 * t + 32,
                                       :])
                        return go
                    return ([mk(hc) for hc in range(HC)] +
                            [mk_dma(hh, t) for hh in range(2)
                             for t in range(2)])

                def v_closures(scp):  # V for sc pair -> one [128,512] bank
                    pp = [None]

                    def mk(half, hc):
                        def go():
                            if half == 0 and hc == 0:
                                pp[0] = psW.tile([P, 512], fp32, tag="pp", name="pp")
                            sc = scp * 2 + half
                            nc.tensor.matmul(
                                pp[0][:, half * HGD:(half + 1) * HGD],
                                xT[:, hc, sc * P:(sc + 1) * P], wv_t[:, hc, :],
                                start=(half == 0 and hc == 0),
                                stop=(hc == HC - 1), skip_group_check=True)
                            if half == 1 and hc == HC - 1:
                                pp2 = pp[0].rearrange("p (s h c) -> p s h c",
                                                      s=2, c=D)
                                bvb = bv4[:, None, :, :].to_broadcast(
                                    [P, 2, NHL, D])
                                nc.vector.tensor_tensor(
                                    Vt8_4[:, scp * 2:scp * 2 + 2, :,
                                          VOFF:VOFF + D],
                                    pp2, bvb, ALU.add)
                                if scp < 2:
                                    nc.vector.tensor_tensor(
                                        Vt16_4[:, scp * 2:scp * 2 + 2, :,
                                               VOFF:VOFF + D],
                                        pp2, bvb, ALU.add)
                        return go
                    return [mk(h, hc) for h in range(2) for hc in range(HC)]

                # ---------- filler queue with dependency markers ----------
                fillers = deque()
                markers = {}
                done = [0]

                def pull(n):
                    for _ in range(n):
                        if not fillers:
                            return
                        fillers.popleft()()
                        done[0] += 1

                def drain_to(marker):
                    tgt = markers.get(marker, 0)
                    while done[0] < tgt:
                        fillers.popleft()()
                        done[0] += 1

                def add_fillers(closures):
                    fillers.extend(closures)

                def set_marker(name):
                    markers[name] = done[0] + len(fillers)

                # ---------- attention ----------
                # et tiles hold an exp'd score PAIR of key blocks:
                # [P, kc%2, head-of-pair, 512q]. fp8 for q rows >= 512
                # (consumed by DoubleRow PV over the kc pair), bf16 for the
                # accuracy-critical first 512 rows (plain per-kc PV).
                def sc_exp(pr, qc, kc, et):
                    # diagonal tiles (j >= 0): columns f < j*128 are fully
                    # masked -> skip them in scores, exp, mask and PV
                    q0 = qc * 512
                    j = kc - qc * 4
                    off = max(0, j) * P
                    sps = psE.tile([P, 1024], fp32, tag="sps", bufs=2, name="sps")
                    if qc == 0:
                        QTa, QTb = QT[0:D, pr, :], QT[D:P, pr, :]
                        KTa, KTb = KT[0:D, pr, :], KT[D:P, pr, :]
                        nc.tensor.matmul(
                            sps[:, off:512], KTa[:, kc * P:(kc + 1) * P],
                            QTa[:, q0 + off:q0 + 512], start=True, stop=True,
                            tile_position=(0, 0))
                        nc.tensor.matmul(
                            sps[:, 512 + off:1024], KTb[:, kc * P:(kc + 1) * P],
                            QTb[:, q0 + off:q0 + 512], start=True, stop=True,
                            tile_position=(64, 0))
                    else:
                        # fp8 DoubleRow: d=64 contraction as two 32-row
                        # k-tiles, 0.5 cycles/row
                        for hh in range(2):
                            b0 = 32 * (2 * pr + hh)
                            nc.tensor.matmul(
                                sps[:, hh * 512 + off:(hh + 1) * 512],
                                K8[b0:b0 + 32, :, kc * P:(kc + 1) * P],
                                Q8[b0:b0 + 32, :, q0 + off:q0 + 512],
                                start=True, stop=True, perf_mode=DR,
                                tile_position=(b0, 0))
                    sps2 = sps.rearrange("p (h f) -> p h f", h=2)
                    bias_t = mask_t if qc == 0 else mask2_t
                    nc.scalar.activation(et[:, kc % 2, :, off:],
                                         sps2[:, :, off:],
                                         AF.Exp, scale=scale,
                                         bias=bias_t[:, kc:kc + 1])
                    if j >= 0:  # zero the partial 128-wide triangle band;
                        # columns >= off+128 of this block are fully unmasked
                        csl = (cm[:, None, j, off:off + P] if qc == 0
                               else cm8[:, None, j, :])
                        nc.vector.tensor_mul(
                            et[:, kc % 2, :, off:off + P],
                            et[:, kc % 2, :, off:off + P],
                            csl.to_broadcast([P, 2, P]))
                    return off

                def pv16(pr, kc, nkc, off, et, ctxa, ctxb):
                    # first 512 q rows: plain bf16 matmul per key block
                    ha, hb = 2 * pr, 2 * pr + 1
                    nc.tensor.matmul(
                        ctxa[:, off:], Vt16_4[:, kc, ha, :],
                        et[:, kc % 2, 0, off:],
                        start=(kc == 0), stop=(kc == nkc - 1))
                    nc.tensor.matmul(
                        ctxb[:, off:], Vt16_4[:, kc, hb, :],
                        et[:, kc % 2, 1, off:],
                        start=(kc == 0), stop=(kc == nkc - 1))

                def pv_dr(pr, qc, kcp, off_pair, et, ctxa, ctxb):
                    # fp8 DoubleRow: both key blocks of the pair in one
                    # matmul per head (0.5 cycles/row)
                    ha, hb = 2 * pr, 2 * pr + 1
                    nkcp = 2 * (qc + 1)
                    nc.tensor.matmul(
                        ctxa[:, off_pair:],
                        Vt8_4[:, 2 * kcp:2 * kcp + 2, ha, :],
                        et[:, :, 0, off_pair:],
                        start=(kcp == 0), stop=(kcp == nkcp - 1),
                        perf_mode=DR)
                    nc.tensor.matmul(
                        ctxb[:, off_pair:],
                        Vt8_4[:, 2 * kcp:2 * kcp + 2, hb, :],
                        et[:, :, 1, off_pair:],
                        start=(kcp == 0), stop=(kcp == nkcp - 1),
                        perf_mode=DR)

                def tail(h, qc, ctx):
                    q0 = qc * 512
                    rcp = rcpp.tile([1, 512], fp32, tag="rcp", name="rcp")
                    nc.vector.reciprocal(rcp[0:1, :], ctx[0:1, :])
                    rb = rcpp.tile([VOFF + D, 512], fp32, tag="rb", name="rb")
                    nc.gpsimd.partition_broadcast(rb[:], rcp[0:1, :])
                    ctxn = cnp.tile([VOFF + D, 512], fp32, tag="cn", name="cn")
                    nc.vector.tensor_mul(ctxn[VOFF:, :], ctx[VOFF:VOFF + D, :],
                                         rb[VOFF:, :])
                    nc.sync.dma_start(
                        out[h * D:(h + 1) * D, q0:q0 + 512], ctxn[VOFF:, :])

                # ---------- schedule ----------
                # prologue block 0: x(sc0..3) -> xT, QT/KT dc0 sq0, V scp0..1
                for sc in range(4):
                    emit_xtr(early_xt[sc], sc, 0)
                    emit_xtr(early_xt[sc], sc, 1)
                for cl in proj_closures(wq_t, bq_t, QT, Q8, 0, 0):
                    cl()
                for cl in proj_closures(wk_t, bk_t, KT, K8, 0, 0):
                    cl()
                # Vt fixed columns (on GPSIMD so they don't block the first
                # xT evacuations in DVE's queue)
                nc.gpsimd.memset(Vt8_4[:, :, :, 0:1], 1.0)
                nc.gpsimd.memset(Vt8_4[:, :, :, 1:VOFF], 0.0)
                nc.gpsimd.memset(Vt16_4[:, :, :, 0:1], 1.0)
                nc.gpsimd.memset(Vt16_4[:, :, :, 1:VOFF], 0.0)

                # V for the first 4 key blocks as fillers so the first score
                # exps reach ACT as early as possible; pv16 drains ("v", kc//2)
                add_fillers(v_closures(0))
                set_marker(("v", 0))
                add_fillers(v_closures(1))
                set_marker(("v", 1))

                # filler blocks 1..3 + C dc1 (+ dc0 later-sq), with markers
                for g in range(1, 4):
                    def blk(g=g):
                        loads, trs = [], []
                        boxes = {}
                        for sc in range(4 * g, 4 * g + 4):
                            boxes[sc] = []

                            def load(sc=sc):
                                boxes[sc].append(emit_xload(sc))

                            loads.append(load)
                            for hg in range(2):
                                def tr(sc=sc, hg=hg):
                                    emit_xtr(boxes[sc][0], sc, hg)
                                trs.append(tr)
                        # 2-deep DMA lookahead: L L t t L t t L t t t t
                        out_cl = [loads[0], loads[1], trs[0], trs[1],
                                  loads[2], trs[2], trs[3],
                                  loads[3], trs[4], trs[5], trs[6], trs[7]]
                        out_cl += proj_closures(wq_t, bq_t, QT, Q8, 0, g)
                        out_cl += proj_closures(wk_t, bk_t, KT, K8, 0, g)
                        return out_cl
                    add_fillers(blk())
                    set_marker(("blk", g))
                    # V behind a finer marker: qc g's scores need not wait
                    # for it; pv drains ("v", kcp) just in time
                    add_fillers(v_closures(2 * g))
                    set_marker(("v", 2 * g))
                    add_fillers(v_closures(2 * g + 1))
                    set_marker(("v", 2 * g + 1))
                for sq in range(QC):
                    add_fillers(proj_closures(wk_t, bk_t, KT, K8, 1, sq))
                for sq in (3, 2, 1, 0):
                    add_fillers(proj_closures(wq_t, bq_t, QT, Q8, 1, sq))
                    set_marker(("cdc1", sq))

                for pr in range(2):
                    qcs = list(range(QC)) if pr == 0 else list(range(QC))[::-1]
                    flat = [(qc, kc) for qc in qcs
                            for kc in range(4 * (qc + 1))]
                    ctxs = {}
                    ets = {}      # (qc, kcp) -> et pair tile
                    offs = {}     # (qc, kc) -> off

                    def ensure(qc):
                        if pr == 0:
                            if qc > 0:
                                drain_to(("blk", qc))
                        else:
                            drain_to(("cdc1", qc))

                    def start_unit(qc):
                        ensure(qc)
                        ctxs[qc] = (
                            psE.tile([VOFF + D, 512], fp32, tag="ctx", bufs=2, name="ctx"),
                            psE.tile([VOFF + D, 512], fp32, tag="ctx", bufs=2, name="ctx"))

                    def emit_scores(qc, kc):
                        kcp = kc // 2
                        if kc % 2 == 0:
                            if qc == 0:
                                et = et16p.tile([P, 2, 2, 512], bf16,
                                                tag="et16", name="et16")
                            else:
                                et = et8p.tile([P, 2, 2, 512], fp8,
                                               tag="et8", name="et8")
                                j_e = kc - qc * 4
                                if j_e >= 0:
                                    # odd member's fully-masked 128-band
                                    # (the DR moving starts at the even off)
                                    nc.gpsimd.memset(
                                        et[:, 1, :, j_e * P:(j_e + 1) * P],
                                        0.0)
                            ets[(qc, kcp)] = et
                        offs[(qc, kc)] = sc_exp(pr, qc, kc, ets[(qc, kcp)])

                    start_unit(flat[0][0])
                    for ahead in range(2):
                        qc, kc = flat[ahead]
                        emit_scores(qc, kc)
                    for i, (qc, kc) in enumerate(flat):
                        nkc = 4 * (qc + 1)
                        kcp = kc // 2
                        if qc == 0:
                            drain_to(("v", kc // 2))
                            pv16(pr, kc, nkc, offs.pop((qc, kc)),
                                 ets[(qc, kcp)], ctxs[qc][0], ctxs[qc][1])
                            if kc % 2 == 1:
                                ets.pop((qc, kcp))
                        elif kc % 2 == 1:
                            off_pair = offs.pop((qc, kc - 1))
                            offs.pop((qc, kc))
                            drain_to(("v", kcp))
                            pv_dr(pr, qc, kcp, off_pair,
                                  ets.pop((qc, kcp)), ctxs[qc][0], ctxs[qc][1])
                        pull(3 if pr == 0 else 1)
                        if i + 2 < len(flat):
                            q2, k2 = flat[i + 2]
                            if k2 == 0:
                                start_unit(q2)
                            emit_scores(q2, k2)
                        if kc == nkc - 1:
                            ca, cb = ctxs.pop(qc)
                            tail(2 * pr, qc, ca)
                            tail(2 * pr + 1, qc, cb)
                # drain any remaining fillers (shouldn't be many)
                while fillers:
                    pull(1)


def build():
    if "nc" not in _CACHE:
        nc = bacc.Bacc("TRN2", target_bir_lowering=False, debug=False,
                       num_devices=NCORES)
        _emit(nc)
        nc.compile()
        _CACHE["nc"] = nc
    return _CACHE["nc"]


def make_in_maps(hidden_states, attention_mask, Wq, bq, Wk, bk, Wv, bv):
    in_maps = []
    for c in range(NCORES):
        b, g = c // 4, c % 4
        sl = slice(g * HGD, (g + 1) * HGD)
        bf = ml_dtypes.bfloat16
        in_maps.append({
            "x": np.ascontiguousarray(hidden_states[b]).astype(bf),
            "wq": np.ascontiguousarray(Wq[:, sl]).astype(bf),
            "wk": np.ascontiguousarray(Wk[:, sl]).astype(bf),
            "wv": np.ascontiguousarray(Wv[:, sl]).astype(bf),
            "bq": np.ascontiguousarray(bq[sl]),
            "bk": np.ascontiguousarray(bk[sl]),
            "bv": np.ascontiguousarray(bv[sl]),
            "mask": np.ascontiguousarray(attention_mask[b, 0, 0, :]),
        })
    return in_maps


def kernel(hidden_states, attention_mask, Wq, bq, Wk, bk, Wv, bv, **run_kwargs):
    global LAST_RESULTS
    hidden_states = np.asarray(hidden_states, dtype=np.float32)
    attention_mask = np.asarray(attention_mask, dtype=np.float32)
    nc = build()
    in_maps = make_in_maps(
        hidden_states, attention_mask,
        np.asarray(Wq, np.float32), np.asarray(bq, np.float32),
        np.asarray(Wk, np.float32), np.asarray(bk, np.float32),
        np.asarray(Wv, np.float32), np.asarray(bv, np.float32))
    res = run_bass_kernel_spmd(nc, in_maps, core_ids=list(range(NCORES)),
                               **run_kwargs)
    LAST_RESULTS = res
    full = np.empty((B, S, H), dtype=np.float32)
    for c in range(NCORES):
        b, g = c // 4, c % 4
        full[b, :, g * HGD:(g + 1) * HGD] = res.results[c]["out"].T
    return full



# revision 57
# speedup vs baseline: 1.1223x; 1.0426x over previous
"""Causal self-attention (B=2, S=2048, H=1024, 16 heads) on 8 trn2 NeuronCores.

Sharding: core c handles batch b = c // 4 and head-group g = c % 4
(4 heads x d=64 = 256 output columns). Fully parallel, no collectives.

v5 pipeline (per core):
  - x, Wq/Wk/Wv loaded in bf16 (halves front-of-kernel HBM traffic);
    x -> xT via bf16 PE transposes (1 cyc/row), one [128,512] DVE evac
  - projections accumulate in fp32 PSUM; evac writes fp8 staging chunks
    (plus a bf16 QT/KT copy of the first 512 q/k rows), then 4 small DMAs
    partition-remap each chunk into the DoubleRow layout:
    Q8/K8[32h+p, t, s] = Q[head h, d=32t+p, s] on partitions 32h..32h+32
  - scores: q rows < 512 via bf16 matmuls (accuracy-critical few-key rows);
    q rows >= 512 via fp8 DoubleRow (two 32-deep d-tiles, 0.5 cyc/row,
    4 heads packed at array row offsets 0/32/64/96)
  - exp on ACT with bias = mask - 3 on the fp8 path: softmax is
    shift-invariant and the shift moves fp8e4 overflow (448) out to ~9
    sigma scores; exp writes fp8/bf16 et pair tiles [P, kc%2, head, 512q]
  - causal diag masking trimmed to the 128-wide partial band (0/1 mask
    mul on DVE); fully-masked odd-pair bands memset on GPSIMD
  - PV: [1|0*63|V] stationary; q<512 plain bf16 per key block; q>=512
    fp8 DoubleRow over key-block PAIRS (0.5 cyc/row); fp32 PSUM ctx
    [128, 512] per head, row 0 = softmax denominator
  - tail: reciprocal (DVE) + partition_broadcast (GPSIMD) + multiply +
    DMA out; output stored TRANSPOSED [256, 2048], host transposes back
  - emission: software-pipelined score stream (2 ahead of PV) with
    projection/V/transpose/remap work woven in as fillers; V work behind
    fine-grained ("v", scp) markers drained just-in-time at PV; mask/bias
    DMAs issued before the 8.7us of W loads; ACT table pre-warmed
"""

from collections import deque

import ml_dtypes
import numpy as np

import concourse.bacc as bacc
import concourse.mybir as mybir
from concourse.tile import TileContext
from concourse.bass_utils import run_bass_kernel_spmd
from concourse.masks import make_identity

B, S, H, NH, D = 2, 2048, 1024, 16, 64
P = 128
NCORES = 8
NHL = NH // 4            # 4 heads per core
HGD = NHL * D            # 256 output cols per core
HC = H // P              # 8 contraction chunks
SC = S // P              # 16 sequence chunks of 128
QC = S // 512            # 4 query chunks of 512
KC = S // P              # 16 key chunks of 128
DC = HGD // P            # 2 partition chunks of QT/KT

fp32 = mybir.dt.float32
f32r = mybir.dt.float32r
bf16 = mybir.dt.bfloat16
fp8 = mybir.dt.float8e4
AF = mybir.ActivationFunctionType
ALU = mybir.AluOpType
DR = mybir.MatmulPerfMode.DoubleRow

_CACHE = {}
LAST_RESULTS = None


def _emit(nc):
    x = nc.declare_dram_parameter("x", [S, H], bf16, isOutput=False)
    wq = nc.declare_dram_parameter("wq", [H, HGD], bf16, isOutput=False)
    wk = nc.declare_dram_parameter("wk", [H, HGD], bf16, isOutput=False)
    wv = nc.declare_dram_parameter("wv", [H, HGD], bf16, isOutput=False)
    bq = nc.declare_dram_parameter("bq", [HGD], fp32, isOutput=False)
    bk = nc.declare_dram_parameter("bk", [HGD], fp32, isOutput=False)
    bv = nc.declare_dram_parameter("bv", [HGD], fp32, isOutput=False)
    mask = nc.declare_dram_parameter("mask", [S], fp32, isOutput=False)
    # transposed output: host does the final [HGD, S] -> [S, HGD] transpose
    out = nc.declare_dram_parameter("out", [HGD, S], fp32, isOutput=True)

    scale = float(1.0 / np.sqrt(np.float32(D)))

    with TileContext(nc) as tc:
        with tc.tile_pool(name="const", bufs=1) as const, \
             tc.tile_pool(name="big", bufs=1) as big:

            # ---- big tiles ----
            xT = big.tile([P, HC, S], bf16, tag="xT")
            # f32r Q/K only for the accuracy-critical first 512 q rows /
            # first 4 key blocks; everything else reads the fp8 copies
            QT = big.tile([P, DC, 512], bf16, tag="QT")
            KT = big.tile([P, DC, 512], bf16, tag="KT")
            VS, VOFF = 128, 64  # per-head [1 | zeros*63 | V] stationary layout
            # fp8 V (all key blocks) for DoubleRow PV on q rows >= 512;
            # bf16 V (key blocks 0..3) for the accuracy-critical first 512 rows
            Vt8 = big.tile([P, SC, NHL * VS], fp8, tag="Vt8")
            Vt8_4 = Vt8.rearrange("p sc (h c) -> p sc h c", c=VS)
            Vt16 = big.tile([P, 4, NHL * VS], bf16, tag="Vt16")
            Vt16_4 = Vt16.rearrange("p sc (h c) -> p sc h c", c=VS)
            # fp8 Q/K in DoubleRow layout for q rows >= 512 scores:
            # head h lives on partitions 32h..32h+32, free = (d-half t, s):
            # Q8[32h+p, t, s] = Q[head h, d=32t+p, s]
            Q8 = big.tile([P, 2, S], fp8, tag="Q8")
            K8 = big.tile([P, 2, S], fp8, tag="K8")

            with tc.tile_pool(name="xin", bufs=16) as xin, \
                 tc.tile_pool(name="et8", bufs=6) as et8p, \
                 tc.tile_pool(name="et16", bufs=3) as et16p, \
                 tc.tile_pool(name="stg", bufs=5) as stgp, \
                 tc.tile_pool(name="rcp", bufs=2) as rcpp, \
                 tc.tile_pool(name="cnorm", bufs=2) as cnp, \
                 tc.tile_pool(name="psW", bufs=2, space="PSUM") as psW, \
                 tc.tile_pool(name="psE", bufs=1, space="PSUM") as psE:

                # ---------- early loads: x first (PE depends on it) ----------
                def emit_xload(sc):
                    xt = xin.tile([P, H], bf16, tag="xt", name="xt")
                    nc.sync.dma_start(xt[:], x[sc * P:(sc + 1) * P, :])
                    return xt

                xt0 = xin.tile([P, H], bf16, tag="xt", name="xt")
                nc.sync.dma_start(xt0[:, 0:512], x[0:P, 0:512])
                nc.sync.dma_start(xt0[:, 512:H], x[0:P, 512:H])
                early_xt = [xt0] + [emit_xload(sc) for sc in range(1, 4)]
                late_xt = {}

                ident_f = const.tile([P, P], fp32, tag="identf")
                make_identity(nc, ident_f)
                ident_r = const.tile([P, P], bf16, tag="identr")
                nc.vector.tensor_copy(ident_r[:], ident_f[:])
                # dummy exp: pulls the 1.28us ACT table load into the idle
                # prologue instead of the first real score exp
                warm = const.tile([1, 1], fp32, tag="warm")
                nc.scalar.activation(warm[0:1, 0:1], ident_f[0:1, 0:1], AF.Exp)
                # 0/1 causal masks: cm[:, j, f] = (f - p >= -j*128)
                cm = const.tile([P, 4, 512], mybir.dt.bfloat16, tag="cmask")
                nc.gpsimd.memset(cm[:], 1.0)
                for j in range(4):
                    nc.gpsimd.affine_select(
                        out=cm[:, j, :], in_=cm[:, j, :],
                        compare_op=ALU.is_ge, fill=0.0,
                        base=-j * P, pattern=[[1, 512]], channel_multiplier=-1)
                # only the 128-wide partial band [j*128, j*128+128) is ever
                # applied; keep an fp8 copy of those bands for the fp8 path
                cm8 = const.tile([P, 4, P], fp8, tag="cmask8")
                for j in range(4):
                    nc.gpsimd.tensor_copy(cm8[:, j, :], cm[:, j, j * P:(j + 1) * P])

                # small bias/mask DMAs first: mask_t gates the very first
                # exp, so it must not queue behind 8.7us of W loads
                bq_t = const.tile([P, DC], fp32, tag="bq")
                bk_t = const.tile([P, DC], fp32, tag="bk")
                nc.sync.dma_start(bq_t[:], bq[:].rearrange("(dc p) -> p dc", p=P))
                nc.sync.dma_start(bk_t[:], bk[:].rearrange("(dc p) -> p dc", p=P))
                mask_t = const.tile([P, KC], fp32, tag="mask")
                nc.sync.dma_start(mask_t[:], mask[:].rearrange("(kc p) -> p kc", p=P))
                bv_b = const.tile([P, HGD], fp32, tag="bv")
                nc.gpsimd.dma_start(bv_b[:], bv[None, :].to_broadcast([P, HGD]))
                bv4 = bv_b.rearrange("p (h c) -> p h c", c=D)
                wq_t = big.tile([P, HC, HGD], bf16, tag="wq")
                wk_t = big.tile([P, HC, HGD], bf16, tag="wk")
                wv_t = big.tile([P, HC, HGD], bf16, tag="wv")
                # W loads on the ACT engine's DGE queue so the x loads
                # (sync queue) are not stuck behind 8.7us of weight traffic
                nc.sync.dma_start(wq_t[:], wq[:].rearrange("(hc p) n -> p hc n", p=P))
                nc.sync.dma_start(wk_t[:], wk[:].rearrange("(hc p) n -> p hc n", p=P))
                nc.sync.dma_start(wv_t[:], wv[:].rearrange("(hc p) n -> p hc n", p=P))
                # eager x: queue every remaining tile now (SBUF is cheap in
                # bf16) so transposes/projections are never DMA-emission-gated
                for sc in range(4, 16):
                    late_xt[sc] = emit_xload(sc)
                # fp8-path exp bias: mask - 3. Softmax is shift-invariant (the
                # ones-column denominator absorbs e^-3), and the shift moves
                # fp8e4 overflow (448) from ~6 sigma scores out to ~9 sigma.
                mask2_t = const.tile([P, KC], fp32, tag="mask2")
                neg3_c = const.tile([P, 1], fp32, tag="neg3")
                nc.vector.memset(neg3_c[:], -3.0)
                nc.vector.tensor_scalar_add(mask2_t[:], mask_t[:],
                                            neg3_c[:, 0:1])

                zero_c = const.tile([P, 1], fp32, tag="zero")
                nc.vector.memset(zero_c[:], 0.0)
                ones_c = const.tile([P, 1], fp32, tag="ones")
                nc.vector.memset(ones_c[:], 1.0)

                def emit_xtr(xt, sc, hg):  # 4 transposes + 1 evac
                    tp = psW.tile([P, 512], bf16, tag="pp", name="tp")
                    for j in range(4):
                        hc = hg * 4 + j
                        nc.tensor.matmul(
                            tp[:, j * P:(j + 1) * P],
                            xt[:, hc * P:(hc + 1) * P], ident_r[:],
                            is_transpose=True, start=(j == 0), stop=(j == 3),
                            skip_group_check=True)
                    nc.vector.tensor_copy(
                        xT[:, hg * 4:(hg + 1) * 4, sc * P:(sc + 1) * P],
                        tp.rearrange("p (j c) -> p j c", c=P))

                def proj_closures(W, bias_t, OUTF, OUT8, dc, sq,
                                  skip8=False):
                    # 8 hc matmuls accumulate in PSUM; evac with bias to fp8
                    # staging (DoubleRow layout via 4 remap DMAs), plus a
                    # bf16 copy of the sq==0 chunk for the q<512 path.
                    # skip8: no fp8 build (Q8's sq0 region is never read)
                    pp = [None]
                    stg = [None]

                    def mk(hc):
                        def go():
                            if hc == 0:
                                pp[0] = psW.tile([P, 512], fp32, tag="pp", name="pp")
                            nc.tensor.matmul(
                                pp[0][:], W[:, hc, dc * P:(dc + 1) * P],
                                xT[:, hc, sq * 512:(sq + 1) * 512],
                                start=(hc == 0), stop=(hc == HC - 1))
                            if hc == HC - 1:
                                if sq == 0:
                                    nc.vector.tensor_scalar_add(
                                        OUTF[:, dc, :],
                                        pp[0][:], bias_t[:, dc:dc + 1])
                                if not skip8:
                                    stg[0] = stgp.tile([P, 512], fp8,
                                                       tag="stg", name="stg")
                                    nc.vector.tensor_scalar_add(
                                        stg[0][:], pp[0][:],
                                        bias_t[:, dc:dc + 1])
                        return go

                    def mk_dma(hh, t):
                        def go():
                            hg = 2 * dc + hh
                            nc.sync.dma_start(
                                OUT8[32 * hg:32 * hg + 32, t,
                                     sq * 512:(sq + 1) * 512],
                                stg[0][64 * hh + 32 * t:64 * hh + 32 * t + 32,
                                       :])
                        return go
                    if skip8:
                        return [mk(hc) for hc in range(HC)]
                    return ([mk(hc) for hc in range(HC)] +
                            [mk_dma(hh, t) for hh in range(2)
                             for t in range(2)])

                def v_closures(scp):  # V for sc pair -> one [128,512] bank
                    pp = [None]

                    def mk(half, hc):
                        def go():
                            if half == 0 and hc == 0:
                                pp[0] = psW.tile([P, 512], fp32, tag="pp", name="pp")
                            sc = scp * 2 + half
                            nc.tensor.matmul(
                                pp[0][:, half * HGD:(half + 1) * HGD],
                                xT[:, hc, sc * P:(sc + 1) * P], wv_t[:, hc, :],
                                start=(half == 0 and hc == 0),
                                stop=(hc == HC - 1), skip_group_check=True)
                            if half == 1 and hc == HC - 1:
                                pp2 = pp[0].rearrange("p (s h c) -> p s h c",
                                                      s=2, c=D)
                                bvb = bv4[:, None, :, :].to_broadcast(
                                    [P, 2, NHL, D])
                                nc.vector.tensor_tensor(
                                    Vt8_4[:, scp * 2:scp * 2 + 2, :,
                                          VOFF:VOFF + D],
                                    pp2, bvb, ALU.add)
                                if scp < 2:
                                    nc.vector.tensor_tensor(
                                        Vt16_4[:, scp * 2:scp * 2 + 2, :,
                                               VOFF:VOFF + D],
                                        pp2, bvb, ALU.add)
                        return go
                    return [mk(h, hc) for h in range(2) for hc in range(HC)]

                # ---------- filler queue with dependency markers ----------
                fillers = deque()
                markers = {}
                done = [0]

                def pull(n):
                    for _ in range(n):
                        if not fillers:
                            return
                        fillers.popleft()()
                        done[0] += 1

                def drain_to(marker):
                    tgt = markers.get(marker, 0)
                    while done[0] < tgt:
                        fillers.popleft()()
                        done[0] += 1

                def add_fillers(closures):
                    fillers.extend(closures)

                def set_marker(name):
                    markers[name] = done[0] + len(fillers)

                # ---------- attention ----------
                # et tiles hold an exp'd score PAIR of key blocks:
                # [P, kc%2, head-of-pair, 512q]. fp8 for q rows >= 512
                # (consumed by DoubleRow PV over the kc pair), bf16 for the
                # accuracy-critical first 512 rows (plain per-kc PV).
                def sc_exp(pr, qc, kc, et):
                    # diagonal tiles (j >= 0): columns f < j*128 are fully
                    # masked -> skip them in scores, exp, mask and PV
                    q0 = qc * 512
                    j = kc - qc * 4
                    off = max(0, j) * P
                    sps = psE.tile([P, 1024], fp32, tag="sps", bufs=2, name="sps")
                    if qc == 0:
                        QTa, QTb = QT[0:D, pr, :], QT[D:P, pr, :]
                        KTa, KTb = KT[0:D, pr, :], KT[D:P, pr, :]
                        nc.tensor.matmul(
                            sps[:, off:512], KTa[:, kc * P:(kc + 1) * P],
                            QTa[:, q0 + off:q0 + 512], start=True, stop=True,
                            tile_position=(0, 0))
                        nc.tensor.matmul(
                            sps[:, 512 + off:1024], KTb[:, kc * P:(kc + 1) * P],
                            QTb[:, q0 + off:q0 + 512], start=True, stop=True,
                            tile_position=(64, 0))
                    else:
                        # fp8 DoubleRow: d=64 contraction as two 32-row
                        # k-tiles, 0.5 cycles/row
                        for hh in range(2):
                            b0 = 32 * (2 * pr + hh)
                            nc.tensor.matmul(
                                sps[:, hh * 512 + off:(hh + 1) * 512],
                                K8[b0:b0 + 32, :, kc * P:(kc + 1) * P],
                                Q8[b0:b0 + 32, :, q0 + off:q0 + 512],
                                start=True, stop=True, perf_mode=DR,
                                tile_position=(b0, 0))
                    sps2 = sps.rearrange("p (h f) -> p h f", h=2)
                    bias_t = mask_t if qc == 0 else mask2_t
                    nc.scalar.activation(et[:, kc % 2, :, off:],
                                         sps2[:, :, off:],
                                         AF.Exp, scale=scale,
                                         bias=bias_t[:, kc:kc + 1])
                    if j >= 0:  # zero the partial 128-wide triangle band;
                        # columns >= off+128 of this block are fully unmasked
                        csl = (cm[:, None, j, off:off + P] if qc == 0
                               else cm8[:, None, j, :])
                        nc.vector.tensor_mul(
                            et[:, kc % 2, :, off:off + P],
                            et[:, kc % 2, :, off:off + P],
                            csl.to_broadcast([P, 2, P]))
                    return off

                def pv16(pr, kc, nkc, off, et, ctxa, ctxb):
                    # first 512 q rows: plain bf16 matmul per key block
                    ha, hb = 2 * pr, 2 * pr + 1
                    nc.tensor.matmul(
                        ctxa[:, off:], Vt16_4[:, kc, ha, :],
                        et[:, kc % 2, 0, off:],
                        start=(kc == 0), stop=(kc == nkc - 1))
                    nc.tensor.matmul(
                        ctxb[:, off:], Vt16_4[:, kc, hb, :],
                        et[:, kc % 2, 1, off:],
                        start=(kc == 0), stop=(kc == nkc - 1))

                def pv_dr(pr, qc, kcp, off_pair, et, ctxa, ctxb):
                    # fp8 DoubleRow: both key blocks of the pair in one
                    # matmul per head (0.5 cycles/row)
                    ha, hb = 2 * pr, 2 * pr + 1
                    nkcp = 2 * (qc + 1)
                    nc.tensor.matmul(
                        ctxa[:, off_pair:],
                        Vt8_4[:, 2 * kcp:2 * kcp + 2, ha, :],
                        et[:, :, 0, off_pair:],
                        start=(kcp == 0), stop=(kcp == nkcp - 1),
                        perf_mode=DR)
                    nc.tensor.matmul(
                        ctxb[:, off_pair:],
                        Vt8_4[:, 2 * kcp:2 * kcp + 2, hb, :],
                        et[:, :, 1, off_pair:],
                        start=(kcp == 0), stop=(kcp == nkcp - 1),
                        perf_mode=DR)

                def tail(h, qc, ctx):
                    q0 = qc * 512
                    rcp = rcpp.tile([1, 512], fp32, tag="rcp", name="rcp")
                    nc.vector.reciprocal(rcp[0:1, :], ctx[0:1, :])
                    rb = rcpp.tile([VOFF + D, 512], fp32, tag="rb", name="rb")
                    nc.gpsimd.partition_broadcast(rb[:], rcp[0:1, :])
                    ctxn = cnp.tile([VOFF + D, 512], fp32, tag="cn", name="cn")
                    nc.vector.tensor_mul(ctxn[VOFF:, :], ctx[VOFF:VOFF + D, :],
                                         rb[VOFF:, :])
                    nc.sync.dma_start(
                        out[h * D:(h + 1) * D, q0:q0 + 512], ctxn[VOFF:, :])

                # ---------- schedule ----------
                # prologue block 0: x(sc0..3) -> xT, QT/KT dc0 sq0.
                # hg0 transposes produce xT hc0-3, so Q's first 4 contraction
                # chunks interleave with the hg1 transposes
                for sc in range(4):
                    emit_xtr(early_xt[sc], sc, 0)
                _q0 = proj_closures(wq_t, bq_t, QT, Q8, 0, 0)
                for cl in _q0[0:4]:
                    cl()
                for sc in range(4):
                    emit_xtr(early_xt[sc], sc, 1)
                for cl in _q0[4:8]:
                    cl()
                # HOLD BACK the remap DMAs: qc0 uses the bf16 path, and these
                # 8 DMAs would sit in the HWDGE FIFO ahead of blk1's x loads
                _k0 = proj_closures(wk_t, bk_t, KT, K8, 0, 0)
                for cl in _k0[:-4]:
                    cl()
                sq0_remaps = _q0[-4:] + _k0[-4:]
                # Vt fixed columns (on GPSIMD so they don't block the first
                # xT evacuations in DVE's queue)
                nc.gpsimd.memset(Vt8_4[:, :, :, 0:1], 1.0)
                nc.gpsimd.memset(Vt8_4[:, :, :, 1:VOFF], 0.0)
                nc.gpsimd.memset(Vt16_4[:, :, :, 0:1], 1.0)
                nc.gpsimd.memset(Vt16_4[:, :, :, 1:VOFF], 0.0)

                # filler blocks 1..3 + C dc1 (+ dc0 later-sq), with markers
                for g in range(1, 4):
                    def blk(g=g):
                        trs = []
                        for sc in range(4 * g, 4 * g + 4):
                            for hg in range(2):
                                def tr(sc=sc, hg=hg):
                                    emit_xtr(late_xt[sc], sc, hg)
                                trs.append(tr)
                        out_cl = list(trs)
                        if g == 1:
                            out_cl += sq0_remaps
                        out_cl += proj_closures(wq_t, bq_t, QT, Q8, 0, g)
                        out_cl += proj_closures(wk_t, bk_t, KT, K8, 0, g)
                        return out_cl
                    add_fillers(blk())
                    set_marker(("blk", g))
                    # dc1 (pr=1) projections as soon as their xT range
                    # exists, ahead of V work, so the pr boundary drain is
                    # small; V is drained just-in-time by trailing PVs
                    if g == 2:
                        add_fillers(proj_closures(wk_t, bk_t, KT, K8, 1, 0))
                        add_fillers(proj_closures(wk_t, bk_t, KT, K8, 1, 1))
                    if g == 3:
                        add_fillers(proj_closures(wk_t, bk_t, KT, K8, 1, 2))
                        add_fillers(proj_closures(wk_t, bk_t, KT, K8, 1, 3))
                        add_fillers(proj_closures(wq_t, bq_t, QT, Q8, 1, 3))
                        set_marker(("cdc1", 3))
                    if g == 1:
                        add_fillers(v_closures(0))
                        set_marker(("v", 0))
                        add_fillers(v_closures(1))
                        set_marker(("v", 1))
                    add_fillers(v_closures(2 * g))
                    set_marker(("v", 2 * g))
                    add_fillers(v_closures(2 * g + 1))
                    set_marker(("v", 2 * g + 1))
                for sq in (2, 1, 0):
                    add_fillers(proj_closures(wq_t, bq_t, QT, Q8, 1, sq))
                    set_marker(("cdc1", sq))

                for pr in range(2):
                    qcs = list(range(QC)) if pr == 0 else list(range(QC))[::-1]
                    flat = [(qc, kc) for qc in qcs
                            for kc in range(4 * (qc + 1))]
                    ctxs = {}
                    ets = {}      # (qc, kcp) -> et pair tile
                    offs = {}     # (qc, kc) -> off

                    def ensure(qc):
                        if pr == 0:
                            if qc > 0:
                                drain_to(("blk", qc))
                        else:
                            drain_to(("cdc1", qc))

                    def start_unit(qc):
                        ensure(qc)
                        ctxs[qc] = (
                            psE.tile([VOFF + D, 512], fp32, tag="ctx", bufs=2, name="ctx"),
                            psE.tile([VOFF + D, 512], fp32, tag="ctx", bufs=2, name="ctx"))

                    def emit_scores(qc, kc):
                        kcp = kc // 2
                        if kc % 2 == 0:
                            if qc == 0:
                                et = et16p.tile([P, 2, 2, 512], bf16,
                                                tag="et16", name="et16")
                            else:
                                et = et8p.tile([P, 2, 2, 512], fp8,
                                               tag="et8", name="et8")
                                j_e = kc - qc * 4
                                if j_e >= 0:
                                    # odd member's fully-masked 128-band
                                    # (the DR moving starts at the even off)
                                    nc.gpsimd.memset(
                                        et[:, 1, :, j_e * P:(j_e + 1) * P],
                                        0.0)
                            ets[(qc, kcp)] = et
                        offs[(qc, kc)] = sc_exp(pr, qc, kc, ets[(qc, kcp)])

                    def do_pv(idx):
                        qc, kc = flat[idx]
                        nkc = 4 * (qc + 1)
                        kcp = kc // 2
                        if qc not in ctxs:
                            ctxs[qc] = (
                                psE.tile([VOFF + D, 512], fp32, tag="ctx",
                                         bufs=2, name="ctx"),
                                psE.tile([VOFF + D, 512], fp32, tag="ctx",
                                         bufs=2, name="ctx"))
                        if qc == 0:
                            drain_to(("v", kc // 2))
                            pv16(pr, kc, nkc, offs.pop((qc, kc)),
                                 ets[(qc, kcp)], ctxs[qc][0], ctxs[qc][1])
                            if kc % 2 == 1:
                                ets.pop((qc, kcp))
                        elif kc % 2 == 1:
                            off_pair = offs.pop((qc, kc - 1))
                            offs.pop((qc, kc))
                            drain_to(("v", kcp))
                            pv_dr(pr, qc, kcp, off_pair,
                                  ets.pop((qc, kcp)), ctxs[qc][0], ctxs[qc][1])
                        pull(3 if pr == 0 else 1)
                        if kc == nkc - 1:
                            ca, cb = ctxs.pop(qc)
                            tail(2 * pr, qc, ca)
                            tail(2 * pr + 1, qc, cb)

                    # decoupled streams: the score/exp cursor leads (ACT is
                    # the pacing engine; scores only need the sps ping-pong),
                    # the PV cursor trails by LAG units so qc-boundary filler
                    # drains never starve ACT. et8 bufs bound the lag.
                    LAG = 9
                    pvd = [0]
                    for i, (qc, kc) in enumerate(flat):
                        if kc == 0:
                            ensure(qc)
                        emit_scores(qc, kc)
                        pull(1)
                        while pvd[0] <= i - LAG:
                            do_pv(pvd[0])
                            pvd[0] += 1
                    while pvd[0] < len(flat):
                        do_pv(pvd[0])
                        pvd[0] += 1
                # drain any remaining fillers (shouldn't be many)
                while fillers:
                    pull(1)


def build():
    if "nc" not in _CACHE:
        nc = bacc.Bacc("TRN2", target_bir_lowering=False, debug=False,
                       num_devices=NCORES)
        _emit(nc)
        nc.compile()
        _CACHE["nc"] = nc
    return _CACHE["nc"]


def make_in_maps(hidden_states, attention_mask, Wq, bq, Wk, bk, Wv, bv):
    in_maps = []
    for c in range(NCORES):
        b, g = c // 4, c % 4
        sl = slice(g * HGD, (g + 1) * HGD)
        bf = ml_dtypes.bfloat16
        in_maps.append({
            "x": np.ascontiguousarray(hidden_states[b]).astype(bf),
            "wq": np.ascontiguousarray(Wq[:, sl]).astype(bf),
            "wk": np.ascontiguousarray(Wk[:, sl]).astype(bf),
            "wv": np.ascontiguousarray(Wv[:, sl]).astype(bf),
            "bq": np.ascontiguousarray(bq[sl]),
            "bk": np.ascontiguousarray(bk[sl]),
            "bv": np.ascontiguousarray(bv[sl]),
            "mask": np.ascontiguousarray(attention_mask[b, 0, 0, :]),
        })
    return in_maps


def kernel(hidden_states, attention_mask, Wq, bq, Wk, bk, Wv, bv, **run_kwargs):
    global LAST_RESULTS
    hidden_states = np.asarray(hidden_states, dtype=np.float32)
    attention_mask = np.asarray(attention_mask, dtype=np.float32)
    nc = build()
    in_maps = make_in_maps(
        hidden_states, attention_mask,
        np.asarray(Wq, np.float32), np.asarray(bq, np.float32),
        np.asarray(Wk, np.float32), np.asarray(bk, np.float32),
        np.asarray(Wv, np.float32), np.asarray(bv, np.float32))
    res = run_bass_kernel_spmd(nc, in_maps, core_ids=list(range(NCORES)),
                               **run_kwargs)
    LAST_RESULTS = res
    full = np.empty((B, S, H), dtype=np.float32)
    for c in range(NCORES):
        b, g = c // 4, c % 4
        full[b, :, g * HGD:(g + 1) * HGD] = res.results[c]["out"].T
    return full

